# revision 64
# baseline (speedup 1.0000x reference)
"""Trainium2 Bass kernel for nn_Center2D (DWT -> pool -> conv-BN-ReLU x2 -> deconv -> IDWT).

Self-contained: hardcodes shapes from the problem spec.
Sharding: pure data parallel, batch dim (8) across 8 cores; BN batch stats
synchronized with a tiny AllReduce (2x128 floats) per BN layer.

Layout strategy per core (one sample):
  io:    x is pre-split on host into even/odd w-columns and cast to f16
         (halves HBM read traffic and makes all front DVE reads
         contiguous); output is stored f16 and upcast on host.
  front: DWT-W as a 6-op ts/tt chain (stt runs at half DVE rate), scaled
         copies offloaded to the ACT engine, 1/D1*1/D2 fold absorbed into
         conv1 weights on host (positive, so max-pools commute); the
         DWT-H/pool wave trails one chunk so DVE never waits on ACT;
         conv1 matmuls issued per row-chunk overlap the front on the
         otherwise-idle PE.
  mid:   conv1 K-packed (ky=0,1 pairs -> 128-deep contraction, via a
         partition-shifted duplicate of the padded input), conv2 as 9
         K-packed matmuls, BN stats via accum_out during PSUM evacuation,
         tiny AllReduce per BN (warmed up by a dummy collective at t=0),
         BN+ReLU applied on DVE in halves so conv2/deconv start early.
  back:  deconv as 4 PE matmuls in h-halves, DRAM round-trip to put H on
         partitions (writes on the SWDGE queue, descriptor-bound
         transposed reads split 4-way over both HWDGE queues), PE matmul
         for IDWT-H with REC2 folded into the banded matrix, IDWT-W as
         ts/tt chains with the terminal odds op on ACT, f16 stores split
         across 16 ExternalOutput tensors.
"""

import os
import numpy as np

import concourse.bass as bass
import concourse.bacc as bacc
import concourse.tile as tile
from concourse import mybir
from concourse.bass_utils import run_bass_kernel_spmd

F32 = mybir.dt.float32
F16 = mybir.dt.float16
AF = mybir.ActivationFunctionType
ALU = mybir.AluOpType

REC = np.array([0.48296291314469025, 0.8365163037378079,
                0.22414386804185735, -0.12940952255092145], dtype=np.float64)
DEC = REC[::-1].copy()

N_CORES = int(os.environ.get("WK_CORES", "8"))
EPS = 1e-5

D0, D1, D2, D3 = (float(DEC[0]), float(DEC[1]), float(DEC[2]), float(DEC[3]))
# DWT-W chain carries 1/D1, DWT-H chain 1/D2 -> fold D1*D2 (positive, so
# the max-pools commute) into conv1 weights
FOLD = D1 * D2


# ---------------------------------------------------------------- host consts
def build_BH():
    """IDWT along one axis as a dense [128, 254] matrix, pre-scaled by REC2
    so the even-column IDWT-W tap needs no temporary."""
    B = np.zeros((128, 254), dtype=np.float64)
    for t in range(127):
        B[t,   2*t] += REC[2]
        B[t+1, 2*t] += REC[0]
        B[t,   2*t+1] += REC[3]
        B[t+1, 2*t+1] += REC[1]
    return (B * REC[2]).astype(np.float32)


def pack_consts(conv1_w, conv2_w, deconv_w, deconv_b,
                bn1_g, bn1_b, bn2_g, bn2_b):
    bhw = build_BH().astype(np.float16)          # [128, 254]

    w1 = conv1_w.astype(np.float64) * FOLD       # fold DWT chain scale
    # packed ky=0/1 pairs: rows 0:64 = ci(ky=0), 64:128 = ci(ky=1)
    w1p = np.zeros((128, 3 * 128), np.float16)
    w1s = np.zeros((64, 3 * 128), np.float16)    # ky=2
    for kx in range(3):
        w1p[0:64, kx*128:(kx+1)*128] = w1[:, :, 0, kx].T
        w1p[64:128, kx*128:(kx+1)*128] = w1[:, :, 1, kx].T
        w1s[:, kx*128:(kx+1)*128] = w1[:, :, 2, kx].T

    w2t = np.zeros((128, 9 * 128), np.float16)
    for ky in range(3):
        for kx in range(3):
            w2t[:, (ky*3+kx)*128:(ky*3+kx+1)*128] = conv2_w[:, :, ky, kx].T

    wdt = np.zeros((128, 4 * 64), np.float16)    # [ci, (k,l,o)]
    for k in range(2):
        for l in range(2):
            wdt[:, (k*2+l)*64:(k*2+l+1)*64] = deconv_w[:, :, k, l]

    return {
        "BHW": bhw,
        "w1p": w1p,
        "w1s": w1s,
        "w2t": w2t,
        "wdt": wdt,
        "db": deconv_b.reshape(64, 1).astype(np.float32),
        "bn1g": bn1_g.reshape(128, 1).astype(np.float32),
        "bn1b": bn1_b.reshape(128, 1).astype(np.float32),
        "bn2g": bn2_g.reshape(128, 1).astype(np.float32),
        "bn2b": bn2_b.reshape(128, 1).astype(np.float32),
    }


# ---------------------------------------------------------------- bass kernel
def build_nc(world=N_CORES, stage=None):
    if stage is None:
        stage = int(os.environ.get("WK_STAGE", "99"))
    nc = bacc.Bacc("TRN2", target_bir_lowering=False)
    use_cc = world > 1

    # x pre-split on host: [c, parity, h, 128] f16 (parity 0 = even w cols)
    x = nc.dram_tensor("x", (64, 2, 256, 128), F16, kind="ExternalInput")
    bhw_d = nc.dram_tensor("BHW", (128, 254), F16, kind="ExternalInput")
    w1p_d = nc.dram_tensor("w1p", (128, 384), F16, kind="ExternalInput")
    w1s_d = nc.dram_tensor("w1s", (64, 384), F16, kind="ExternalInput")
    w2t_d = nc.dram_tensor("w2t", (128, 1152), F16, kind="ExternalInput")
    wdt_d = nc.dram_tensor("wdt", (128, 256), F16, kind="ExternalInput")
    db_d = nc.dram_tensor("db", (64, 1), F32, kind="ExternalInput")
    bn_vecs = {n: nc.dram_tensor(n, (128, 1), F32, kind="ExternalInput")
               for n in ("bn1g", "bn1b", "bn2g", "bn2b")}
    # 16 separate f16 output tensors (h-slices); host upcasts to f32
    OUT_SPLITS = []
    h0 = 0
    for i in range(16):
        sz = 16 if (i % 8) != 7 else 15
        OUT_SPLITS.append((h0, sz))
        h0 += sz
    out_ds = [nc.dram_tensor(f"out{i}", (sz, 64, 254), F16, kind="ExternalOutput")
              for i, (_, sz) in enumerate(OUT_SPLITS)]

    scr2 = nc.dram_tensor("scr2", (64, 128, 128), F16, kind="Internal")
    cc_bufs = []
    for i in (0, 1, 2):
        cc_bufs.append((
            nc.dram_tensor(f"bn{i}_in", (128, 2), F32, kind="Internal"),
            nc.dram_tensor(f"bn{i}_out", (128, 2), F32, kind="Internal",
                           addr_space="Shared"),
        ))
    rg = [list(range(world))]
    cnt = float(world * 64 * 64)

    with tile.TileContext(nc) as tc, \
         tc.tile_pool(name="persist", bufs=1) as pp:
        def _body():
            # warmup collective: absorbs the ~11us first-call ncfw setup
            if use_cc:
                nc.gpsimd.collective_compute(
                    "AllReduce", ALU.add, replica_groups=rg,
                    ins=[cc_bufs[0][0][:]], outs=[cc_bufs[0][1][:]])

            # x viewed as [(g c) -> 128 partitions, h_local, w2]; partition
            # group g=0 holds h 0:128, g=1 holds h 128:256 of channel c
            xg = x[:].rearrange("c p (g r) w -> g p c (r w)", g=2)  # [2,2,64,128*128]

            # ---------- chunk 0 loads first (heads the dependency graph)
            xin_pool = tc.tile_pool(name="xin", bufs=2)
            xin = xin_pool.__enter__()
            xe0 = xin.tile([128, 32 * 128], F16, tag="xe")
            xo0 = xin.tile([128, 32 * 128], F16, tag="xo")
            # chunk 0 in 16-row pieces so the first DVE op starts sooner
            for c0 in (0, 2048):
                nc.sync.dma_start(xe0[0:64, c0:c0+2048],
                                  xg[0, 0, :, c0:c0+2048])
                nc.scalar.dma_start(xe0[64:128, c0:c0+2048],
                                    xg[1, 0, :, c0:c0+2048])
                nc.sync.dma_start(xo0[0:64, c0:c0+2048],
                                  xg[0, 1, :, c0:c0+2048])
                nc.scalar.dma_start(xo0[64:128, c0:c0+2048],
                                    xg[1, 1, :, c0:c0+2048])

            # conv1 input, with partition-shifted duplicate for K-packing:
            # parts 0:64 row r = padded row r; parts 64:128 row r = padded r+1
            in1b = pp.tile([128, 66 * 66], F16, name="in1b")
            nc.gpsimd.memset(in1b[:], 0.0)
            p1v = in1b[:].rearrange("p (r v) -> p r v", v=66)

            # ---------------- front: DWT + pool on DVE/GpSimd ----------
            front_pool = tc.tile_pool(name="front", bufs=1)
            fp = front_pool.__enter__()

            y_t = fp.tile([128, 128 * 128], F16, name="y_t")    # DWT-W out (y')
            y_v = y_t[:].rearrange("p (h t) -> p h t", t=128)
            y2_t = fp.tile([128, 64 * 128], F16, name="y2_t")   # DWT-H out (y2'')
            y2_v = y2_t[:].rearrange("p (s t) -> p s t", t=128)
            pw_t = fp.tile([128, 64 * 64], F16, name="pw_t")    # pool-W out
            pw_v = pw_t[:].rearrange("p (s u) -> p s u", u=64)

            # consts to SBUF (after chunk-0 loads in queue order; tiny ones
            # go on the gpsimd SWDGE queue so they don't delay the scalar
            # queue's first front ops)
            w1p_sb = pp.tile([128, 384], F16, name="w1p_sb")
            nc.sync.dma_start(w1p_sb[:], w1p_d[:])
            w1s_sb = pp.tile([64, 384], F16, name="w1s_sb")
            nc.gpsimd.dma_start(w1s_sb[:], w1s_d[:])
            bnv = {}
            for n, d in bn_vecs.items():
                t = pp.tile([128, 1], F32, name=f"{n}_sb")
                nc.gpsimd.dma_start(t[:], d[:])
                bnv[n] = t

            # conv1 state (PSUM banks live across the whole front)
            a1_sb = pp.tile([128, 4096], F16, name="a1_sb")
            junk = pp.tile([128, 512], F32, name="junk")
            s1b = pp.tile([128, 8], F32, name="s1b")
            s2b = pp.tile([128, 8], F32, name="s2b")
            a1v = a1_sb[:].rearrange("p (r q) -> p r q", q=64)
            psB_pool = tc.tile_pool(name="psB", bufs=8, space="PSUM")
            psB = psB_pool.__enter__()
            ps_list = [psB.tile([128, 512], F32, tag="psB", name=f"c1ps{i}")
                       for i in range(8)]

            def conv1_chunk(ch):
                q0 = ch * 8
                for kx in range(3):
                    nc.tensor.matmul(ps_list[ch][:],
                                     w1s_sb[:, kx*128:(kx+1)*128],
                                     p1v[0:64, q0+2:q0+10, kx:kx+64],
                                     start=(kx == 0), stop=False)
                for kx in range(3):
                    nc.tensor.matmul(ps_list[ch][:],
                                     w1p_sb[:, kx*128:(kx+1)*128],
                                     p1v[:, q0:q0+8, kx:kx+64],
                                     start=False, stop=(kx == 2))
                # evacuate + BN1 stats (conv bias is a no-op before BN)
                nc.vector.tensor_scalar(a1v[:, q0:q0+8, :], ps_list[ch][:],
                                        1.0, 0.0, ALU.mult, ALU.add,
                                        accum_out=s1b[:, ch:ch+1])
                nc.scalar.activation(junk[:], ps_list[ch][:], AF.Square,
                                     accum_out=s2b[:, ch:ch+1])

            r30, r10, r02 = D3 / D2, D1 / D0, D0 / D2
            m0, m1 = (D1 + D2) / D2, (D0 + D3) / D2
            # DWT-W ts/tt chain ratios (stt runs at half DVE rate, so cheap
            # ops beat 3 stt): y'' = y/D1. The host pre-scales the O plane
            # by D0/D1, so u2 needs no trailing scale.
            w_s1, w_s2 = D3 / D1, D2 / D0
            mw0 = (D1 + D2) / D1
            mw1 = ((D0 + D3) / D1) * (D1 / D0)   # for the D0/D1-scaled O

            def dwt_h_piece(a, b, v1_p, v2_p):
                """y2''[s] for s_local in [a, b) (a >= 1), both part groups.
                v1 = (d3/d2) y'[2s-2] + y'[2s-1]; v2 = (d1/d0) y'[2s] + y'[2s+1]
                y2'' = (d0/d2) v2 + v1; scaled copies on ACT, adds on DVE"""
                n = b - a
                v1 = v1_p[:].rearrange("p (s t) -> p s t", t=128)[:, 0:n, :]
                v2 = v2_p[:].rearrange("p (s t) -> p s t", t=128)[:, 0:n, :]
                nc.vector.scalar_tensor_tensor(v1, y_v[:, 2*a-2:2*b-3:2, :],
                                               r30, y_v[:, 2*a-1:2*b-2:2, :],
                                               ALU.mult, ALU.add)
                nc.vector.scalar_tensor_tensor(v2, y_v[:, 2*a:2*b-1:2, :],
                                               r10, y_v[:, 2*a+1:2*b:2, :],
                                               ALU.mult, ALU.add)
                nc.vector.scalar_tensor_tensor(y2_v[:, a:b, :], v2, r02, v1,
                                               ALU.mult, ALU.add)

            conv1_done = set()

            def issue_conv1(chs):
                for ch in chs:
                    if ch not in conv1_done:
                        conv1_done.add(ch)
                        conv1_chunk(ch)

            with tc.tile_pool(name="twp", bufs=2) as twp, \
                 tc.tile_pool(name="thp", bufs=2) as thp:

                def wave(hc):
                    # DWT-H + pools + assembly for the s-range chunk hc
                    # completed; issued one chunk late so these DVE ops run
                    # while ACT produces the next chunk's scaled copies
                    a = max(1, 16 * hc)
                    b = 16 * hc + 16
                    v1_p = thp.tile([128, 16 * 128], F16, tag="v1")
                    v2_p = thp.tile([128, 16 * 128], F16, tag="v2")
                    dwt_h_piece(a, b, v1_p, v2_p)
                    if hc == 0:
                        # s_local=0, group 0 mirror: m0 y'[0] + m1 y'[1]
                        nc.vector.scalar_tensor_tensor(
                            y2_v[0:64, 0:1, :], y_v[0:64, 0:1, :], m0 / m1,
                            y_v[0:64, 1:2, :], ALU.mult, ALU.add)
                        nc.vector.tensor_scalar(y2_v[0:64, 0:1, :],
                                                y2_v[0:64, 0:1, :],
                                                m1, None, ALU.mult)
                        nc.vector.tensor_tensor(pw_v[0:64, 0:1, :],
                                                y2_v[0:64, 0:1, 0::2],
                                                y2_v[0:64, 0:1, 1::2], ALU.max)
                    # pool-W for those s rows (on the otherwise-idle Pool eng)
                    nc.vector.tensor_tensor(pw_v[:, a:b, :],
                                            y2_v[:, a:b, 0::2],
                                            y2_v[:, a:b, 1::2], ALU.max)
                    if hc == 3:
                        # s_local=0 group 1 seam: needs y' rows 126,127 of g0
                        seam = fp.tile([128, 2 * 128], F16, name="seam")
                        nc.sync.dma_start(seam[64:128, :],
                                          y_t[0:64, 126*128:128*128])
                        seam_v = seam[:].rearrange("p (h t) -> p h t", t=128)
                        sv1 = fp.tile([128, 128], F16, name="sv1")
                        sv2 = fp.tile([128, 128], F16, name="sv2")
                        nc.vector.scalar_tensor_tensor(
                            sv1[64:128, :].rearrange("p (a t) -> p a t", a=1),
                            seam_v[64:128, 0:1, :], r30,
                            seam_v[64:128, 1:2, :], ALU.mult, ALU.add)
                        nc.vector.scalar_tensor_tensor(
                            sv2[64:128, :].rearrange("p (a t) -> p a t", a=1),
                            y_v[64:128, 0:1, :], r10,
                            y_v[64:128, 1:2, :], ALU.mult, ALU.add)
                        nc.vector.scalar_tensor_tensor(
                            y2_v[64:128, 0:1, :],
                            sv2[64:128, :].rearrange("p (a t) -> p a t", a=1),
                            r02,
                            sv1[64:128, :].rearrange("p (a t) -> p a t", a=1),
                            ALU.mult, ALU.add)
                        nc.vector.tensor_tensor(pw_v[64:128, 0:1, :],
                                                y2_v[64:128, 0:1, 0::2],
                                                y2_v[64:128, 0:1, 1::2],
                                                ALU.max)
                        # deferred pooled row q=32 (block1 row 32) + its
                        # cross-partition copy into block0 row 33
                        nc.vector.tensor_tensor(p1v[64:128, 32:33, 1:65],
                                                pw_v[64:128, 0:1, :],
                                                pw_v[64:128, 1:2, :], ALU.max)
                        nc.sync.dma_start(p1v[0:64, 33:34, :],
                                          p1v[64:128, 32:33, :])
                    # pool-H for pooled rows this chunk completes:
                    # g0: q rows 8hc..8hc+7 -> p1 block0 rows 1+8hc..8+8hc
                    # g1: q rows 32+8hc..39+8hc -> p1 block1 rows 32+8hc..
                    # (g1 pooled row 32 needs the hc==3 seam -> deferred,
                    #  so at hc==0 only rows 33..39 are written here)
                    qa = 8 * hc
                    src0 = pw_v[0:64, 2*qa:2*qa+16, :]
                    nc.vector.tensor_tensor(p1v[0:64, 1+qa:9+qa, 1:65],
                                            src0[:, 0::2, :], src0[:, 1::2, :],
                                            ALU.max)
                    r1a = 33 if hc == 0 else 32 + qa    # block1 first row
                    src1 = pw_v[64:128, 2*(r1a-32):2*qa+16, :]
                    nc.vector.tensor_tensor(p1v[64:128, r1a:40+qa, 1:65],
                                            src1[:, 0::2, :], src1[:, 1::2, :],
                                            ALU.max)
                    # cross-partition copies for the packed layout
                    # block0 rows r1a+1..40+qa <- block1 rows r1a..39+qa
                    nc.sync.dma_start(p1v[0:64, r1a+1:41+qa, :],
                                      p1v[64:128, r1a:40+qa, :])
                    # block1 rows qa..qa+7 <- block0 rows 1+qa..8+qa
                    nc.scalar.dma_start(p1v[64:128, qa:qa+8, :],
                                        p1v[0:64, 1+qa:9+qa, :])
                    # conv1 chunks that just became ready
                    if stage > 2:
                        if hc == 1:
                            issue_conv1([0])
                        elif hc == 2:
                            issue_conv1([1, 5])

                for hc in range(4):          # h-chunks of 32 rows
                    if hc == 0:
                        xe, xo = xe0, xo0
                    else:
                        xe = xin.tile([128, 32 * 128], F16, tag="xe")
                        xo = xin.tile([128, 32 * 128], F16, tag="xo")
                        o0 = hc * 4096
                        nc.sync.dma_start(xe[0:64], xg[0, 0, :, o0:o0+4096])
                        nc.scalar.dma_start(xe[64:128], xg[1, 0, :, o0:o0+4096])
                        nc.sync.dma_start(xo[0:64], xg[0, 1, :, o0:o0+4096])
                        nc.scalar.dma_start(xo[64:128], xg[1, 1, :, o0:o0+4096])
                    # previous chunk's DWT-H/pool wave first: its DVE ops are
                    # ready now and fill the DVE while ACT produces this
                    # chunk's scaled copies
                    if hc >= 1:
                        wave(hc - 1)
                    ev = xe[:].rearrange("p (h w) -> p h w", w=128)
                    ov = xo[:].rearrange("p (h w) -> p h w", w=128)
                    yc = y_v[:, hc*32:(hc+1)*32, :]
                    # DWT-W 5-op chain (y'' = y/D1, O plane pre-scaled by
                    # D0/D1 on host): u1 = (D3/D1) E' + E''
                    # u2 = (D2/D0) Os' + Os'';  y'' = u1 + u2
                    # Scaled copies go to the idle ACT engine (except chunk
                    # 0, where ACT would delay the DVE start); chunk 0 runs
                    # in 16-row sub-chunks to shorten the lead-in.
                    for r0, rn in (((0, 16), (16, 16)) if hc == 0
                                   else ((0, 32),)):
                        evr = ev[:, r0:r0+rn, :]
                        ovr = ov[:, r0:r0+rn, :]
                        ycr = yc[:, r0:r0+rn, :]
                        u1 = twp.tile([128, 32 * 127], F16, tag="u1")
                        u2 = twp.tile([128, 32 * 127], F16, tag="u2")
                        u1v = u1[:].rearrange("p (h t) -> p h t",
                                              t=127)[:, 0:rn, :]
                        u2v = u2[:].rearrange("p (h t) -> p h t",
                                              t=127)[:, 0:rn, :]
                        if hc == 0:
                            nc.vector.tensor_scalar(u1v, evr[:, :, 0:127],
                                                    w_s1, None, ALU.mult)
                            nc.vector.tensor_scalar(u2v, ovr[:, :, 0:127],
                                                    w_s2, None, ALU.mult)
                        else:
                            nc.scalar.activation(u1v, evr[:, :, 0:127],
                                                 AF.Identity, scale=w_s1)
                            nc.scalar.activation(u2v, ovr[:, :, 0:127],
                                                 AF.Identity, scale=w_s2)
                        nc.vector.tensor_tensor(u1v, u1v, evr[:, :, 1:128],
                                                ALU.add)
                        nc.vector.tensor_tensor(u2v, u2v, ovr[:, :, 1:128],
                                                ALU.add)
                        nc.vector.tensor_tensor(ycr[:, :, 1:128], u1v, u2v,
                                                ALU.add)
                        # t=0 mirror: y''[0] = mw0 E[0] + mw1 Os[0]
                        nc.vector.scalar_tensor_tensor(
                            ycr[:, :, 0:1],
                            evr[:, :, 0:1], mw0 / mw1, ovr[:, :, 0:1],
                            ALU.mult, ALU.add)
                        nc.vector.tensor_scalar(ycr[:, :, 0:1],
                                                ycr[:, :, 0:1],
                                                mw1, None, ALU.mult)
                wave(3)
                # mid/back consts (queues free once front loads are done)
                w2t_sb = pp.tile([128, 1152], F16, name="w2t_sb")
                nc.sync.dma_start(w2t_sb[:], w2t_d[:])
                wdt_sb = pp.tile([128, 256], F16, name="wdt_sb")
                nc.scalar.dma_start(wdt_sb[:], wdt_d[:])
                db_sb = pp.tile([64, 1], F32, name="db_sb")
                nc.sync.dma_start(db_sb[:], db_d[:])
                bhw_sb = pp.tile([128, 254], F16, name="bhw_sb")
                nc.scalar.dma_start(bhw_sb[:], bhw_d[:])

            front_pool.__exit__(None, None, None)
            xin_pool.__exit__(None, None, None)
            if stage <= 2:
                psB_pool.__exit__(None, None, None)
                return

            # ---------------- conv1 rest (+BN1) ----------------
            mid_pool = tc.tile_pool(name="mid", bufs=1)
            mp = mid_pool.__enter__()
            in2_pad = mp.tile([128, 66 * 66], F16, name="in2_pad")
            nc.gpsimd.memset(in2_pad[:], 0.0)
            issue_conv1([2, 3, 4, 6, 7])
            psB_pool.__exit__(None, None, None)

            if stage <= 3:
                mid_pool.__exit__(None, None, None)
                return
            sc1, bi1 = _bn_coeffs(nc, pp, s1b, s2b, cc_bufs[1], rg, cnt,
                                  bnv["bn1g"], bnv["bn1b"], use_cc, tag=1)

            # BN1 + ReLU on DVE (faster than ACT, and DVE is idle here),
            # split in h-halves so conv2 chunks can start early
            p2v = in2_pad[:].rearrange("p (r v) -> p r v", v=66)

            def bn1_apply(hh):
                dst = p2v[:, 1+32*hh:33+32*hh, 1:65]
                nc.vector.tensor_scalar(dst, a1v[:, 32*hh:32*hh+32, :],
                                        sc1[:], bi1[:], ALU.mult, ALU.add)
                nc.vector.tensor_scalar_max(dst, dst, 0.0)

            if stage <= 4:
                bn1_apply(0)
                bn1_apply(1)
                mid_pool.__exit__(None, None, None)
                return
            # ---------------- conv2 (+BN2 stats) ----------------
            h2_sb = mp.tile([128, 4096], F16, name="h2_sb")
            h2v = h2_sb[:].rearrange("p (r q) -> p r q", q=64)
            s1c = pp.tile([128, 8], F32, name="s1c")
            s2c = pp.tile([128, 8], F32, name="s2c")
            with tc.tile_pool(name="psC", bufs=8, space="PSUM") as psC:
                ps_list2 = [psC.tile([128, 512], F32, tag="psC", name=f"c2ps{i}")
                            for i in range(8)]

                def conv2_chunk(ch):
                    p0 = ch * 8
                    for ti in range(9):
                        ky, kx = divmod(ti, 3)
                        rhs = p2v[:, p0+ky:p0+ky+8, kx:kx+64]
                        nc.tensor.matmul(ps_list2[ch][:],
                                         w2t_sb[:, ti*128:(ti+1)*128],
                                         rhs, start=(ti == 0), stop=(ti == 8))
                    nc.vector.tensor_scalar(h2v[:, p0:p0+8, :],
                                            ps_list2[ch][:],
                                            1.0, 0.0, ALU.mult, ALU.add,
                                            accum_out=s1c[:, ch:ch+1])
                    nc.scalar.activation(junk[:], ps_list2[ch][:], AF.Square,
                                         accum_out=s2c[:, ch:ch+1])

                bn1_apply(0)
                for ch in (0, 1, 2):
                    conv2_chunk(ch)
                bn1_apply(1)
                for ch in (3, 4, 5, 6, 7):
                    conv2_chunk(ch)

            sc2, bi2 = _bn_coeffs(nc, pp, s1c, s2c, cc_bufs[2], rg, cnt,
                                  bnv["bn2g"], bnv["bn2b"], use_cc, tag=2)

            def bn2_apply(r0, rn):
                dst = h2v[:, r0:r0+rn, :]
                nc.vector.tensor_scalar(dst, dst, sc2[:], bi2[:],
                                        ALU.mult, ALU.add)
                nc.vector.tensor_scalar_max(dst, dst, 0.0)

            if stage <= 5:
                bn2_apply(0, 64)
                mid_pool.__exit__(None, None, None)
                return
            # ---------------- deconv ----------------
            dth = pp.tile([128, 64 * 128], F16, name="dth")
            dth_v = dth[:].rearrange("p (o w) -> p o w", w=128)
            scr2_h = scr2[:].rearrange("o h w -> h o w")
            d_sb = mp.tile([64, 128 * 128], F16, name="d_sb")
            dv = d_sb[:].rearrange("p (h w) -> p h w", w=128)
            with tc.tile_pool(name="psD", bufs=8, space="PSUM") as psD:
                # scr2 writes ride the SWDGE queue; the descriptor-bound
                # transposed reads get both HWDGE queues, split 4-way so all
                # 16 SDMA engines stay fed
                for r0, rn in ((0, 32), (32, 32)):
                    bn2_apply(r0, rn)
                    for kl in range(4):
                        k, l = divmod(kl, 2)
                        for ch in range(rn // 8):
                            p0 = r0 + ch * 8
                            ps = psD.tile([64, 512], F32, tag="psD")
                            nc.tensor.matmul(ps[:], wdt_sb[:, kl*64:(kl+1)*64],
                                             h2v[:, p0:p0+8, :],
                                             start=True, stop=True)
                            dst = dv[:, 2*p0+k:2*p0+k+15:2, l::2]
                            if (kl * 4 + ch) % 2 == 0:
                                nc.vector.tensor_scalar(dst, ps[:], 1.0,
                                                        db_sb[:],
                                                        ALU.mult, ALU.add)
                            else:
                                nc.scalar.activation(dst, ps[:], AF.Identity,
                                                     bias=db_sb[:], scale=1.0)
                    h0, hn = 2 * r0, 2 * rn
                    nc.gpsimd.dma_start(scr2[:, h0:h0+hn, :],
                                        dv[:, h0:h0+hn, :])
                    for oq in range(4):
                        eng = nc.sync if oq % 2 == 0 else nc.scalar
                        eng.dma_start(dth_v[h0:h0+hn, oq*16:(oq+1)*16, :],
                                      scr2_h[h0:h0+hn, oq*16:(oq+1)*16, :])
            mid_pool.__exit__(None, None, None)
            if stage <= 6:
                return

            # ---------------- IDWT-H on PE, IDWT-W spread ----------------
            # g2 = REC2 * (IDWT-H of dth)  (REC2 folded into BHW)
            # out evens: o[2t] = g2[t] + (REC0/REC2) g2[t+1]   (one stt)
            # out odds:  o[2t+1] = (REC3/REC2) g2[t] + (REC1/REC2) g2[t+1]
            q0r, q1r, q3r = REC[0] / REC[2], REC[1] / REC[2], REC[3] / REC[2]
            with tc.tile_pool(name="psE", bufs=8, space="PSUM") as psE, \
                 tc.tile_pool(name="gpool", bufs=2) as gpool, \
                 tc.tile_pool(name="twpool", bufs=2) as twpool, \
                 tc.tile_pool(name="opool", bufs=3) as opool:
                for blk in range(2):
                    g_t = gpool.tile([127, 8192], F16, tag="g")
                    g_v = g_t[:].rearrange("p (o w) -> p o w", w=128)
                    for nch in range(16):
                        ps = psE.tile([127, 512], F32, tag="psE")
                        nc.tensor.matmul(ps[:], bhw_sb[:, blk*127:blk*127+127],
                                         dth[:, nch*512:(nch+1)*512],
                                         start=True, stop=True)
                        dst = g_t[:, nch*512:(nch+1)*512]
                        if nch % 2 == 0:
                            nc.vector.tensor_copy(dst, ps[:])
                        else:
                            nc.scalar.copy(dst, ps[:])
                    # blk1 split finer so the final store tail is short
                    pieces = ((0, 32), (32, 32)) if blk == 0 else \
                             ((0, 32), (32, 16), (48, 16))
                    for pi, (r0, rn) in enumerate(pieces):
                        gh = g_v[:, r0:r0+rn, :]
                        o_t = opool.tile([127, 32 * 254], F16, tag="o")
                        o_v = o_t[:].rearrange("p (o w) -> p o w",
                                               w=254)[:, 0:rn, :]
                        # ts/tt chains (stt runs at half DVE rate); the
                        # terminal odds op goes to ACT so nothing on the DVE
                        # critical path waits for the slower engine
                        # evens: o[2t] = q0r g2[t+1] + g2[t]
                        ae = twpool.tile([127, 32 * 127], F16, tag="ae")
                        aev = ae[:].rearrange("p (o w) -> p o w",
                                              w=127)[:, 0:rn, :]
                        nc.vector.tensor_scalar(aev, gh[:, :, 1:128], q0r,
                                                None, ALU.mult)
                        nc.vector.tensor_tensor(o_v[:, :, 0:253:2], aev,
                                                gh[:, :, 0:127], ALU.add)
                        # odds: o[2t+1] = q3r ((q1r/q3r) g2[t+1] + g2[t])
                        ao = twpool.tile([127, 32 * 127], F16, tag="ao")
                        aov = ao[:].rearrange("p (o w) -> p o w",
                                              w=127)[:, 0:rn, :]
                        nc.vector.tensor_scalar(aov, gh[:, :, 1:128], q1r/q3r,
                                                None, ALU.mult)
                        nc.vector.tensor_tensor(aov, aov, gh[:, :, 0:127],
                                                ALU.add)
                        nc.scalar.activation(o_v[:, :, 1:254:2], aov,
                                             AF.Identity, scale=q3r)
                        for i in range(8):
                            oi = blk * 8 + i
                            h0, sz = OUT_SPLITS[oi]
                            p0 = h0 - blk * 127
                            eng = nc.sync if (i + pi) % 2 == 0 else nc.scalar
                            eng.dma_start(out_ds[oi][:, r0:r0+rn, :],
                                          o_v[p0:p0+sz])

        _body()
    nc.compile()
    return nc


def _bn_coeffs(nc, pp, s1b, s2b, cc_pair, rg, cnt, g_sb, b_sb, use_cc, tag):
    """Reduce per-chunk sums, AllReduce across cores, return (scale, bias) [128,1]."""
    ALU = mybir.AluOpType
    sl = pp.tile([128, 2], F32, name=f"bn{tag}_sl")
    nc.vector.tensor_reduce(sl[:, 0:1], s1b[:], mybir.AxisListType.X, ALU.add)
    nc.vector.tensor_reduce(sl[:, 1:2], s2b[:], mybir.AxisListType.X, ALU.add)
    cc_in, cc_out = cc_pair
    sg = pp.tile([128, 2], F32, name=f"bn{tag}_sg")
    if use_cc:
        nc.sync.dma_start(cc_in[:], sl[:])
        nc.gpsimd.collective_compute(
            "AllReduce", ALU.add, replica_groups=rg,
            ins=[cc_in[:]], outs=[cc_out[:]])
        nc.sync.dma_start(sg[:], cc_out[:])
    else:
        nc.vector.tensor_copy(sg[:], sl[:])

    m = pp.tile([128, 1], F32, name=f"bn{tag}_m")
    vpe = pp.tile([128, 1], F32, name=f"bn{tag}_v")
    t0 = pp.tile([128, 1], F32, name=f"bn{tag}_t0")
    nc.vector.tensor_scalar(m[:], sg[:, 0:1], 1.0 / cnt, None, ALU.mult)
    nc.vector.tensor_tensor(t0[:], m[:], m[:], ALU.mult)          # m^2
    nc.vector.tensor_scalar(vpe[:], sg[:, 1:2], 1.0 / cnt, float(EPS), ALU.mult,
                            ALU.add)                              # E[x^2]+eps
    nc.vector.tensor_tensor(vpe[:], vpe[:], t0[:], ALU.subtract)  # var+eps
    # rsqrt with one Newton step (ACT Sqrt is low-precision)
    s0 = pp.tile([128, 1], F32, name=f"bn{tag}_s0")
    y0 = pp.tile([128, 1], F32, name=f"bn{tag}_y0")
    nc.scalar.activation(s0[:], vpe[:], mybir.ActivationFunctionType.Sqrt)
    nc.vector.reciprocal(y0[:], s0[:])
    t1 = pp.tile([128, 1], F32, name=f"bn{tag}_t1")
    nc.vector.tensor_tensor(t1[:], y0[:], y0[:], ALU.mult)
    nc.vector.tensor_tensor(t1[:], t1[:], vpe[:], ALU.mult)
    nc.vector.tensor_scalar(t1[:], t1[:], -0.5, 1.5, ALU.mult, ALU.add)
    nc.vector.tensor_tensor(y0[:], y0[:], t1[:], ALU.mult)        # refined rsqrt
    sc = pp.tile([128, 1], F32, name=f"bn{tag}_sc")
    bi = pp.tile([128, 1], F32, name=f"bn{tag}_bi")
    nc.vector.tensor_tensor(sc[:], y0[:], g_sb[:], ALU.mult)
    nc.vector.tensor_tensor(t0[:], m[:], sc[:], ALU.mult)
    nc.vector.tensor_tensor(bi[:], b_sb[:], t0[:], ALU.subtract)
    return sc, bi


# ---------------------------------------------------------------- entry point
_CACHE = {}


def kernel(x, conv1_w, conv1_b, bn1_g, bn1_b, conv2_w, conv2_b, bn2_g, bn2_b,
           deconv_w, deconv_b):
    world = N_CORES
    if "nc" not in _CACHE:
        _CACHE["nc"] = build_nc(world)
    nc = _CACHE["nc"]

    consts = pack_consts(np.asarray(conv1_w), np.asarray(conv2_w),
                         np.asarray(deconv_w), np.asarray(deconv_b),
                         np.asarray(bn1_g), np.asarray(bn1_b),
                         np.asarray(bn2_g), np.asarray(bn2_b))
    x = np.asarray(x)
    # host-side prep: f16 + even/odd w split -> [c, parity, h, 128];
    # the odd plane carries the D0/D1 factor of the DWT-W chain
    xs = np.stack([x[:, :, :, 0::2].astype(np.float16),
                   (x[:, :, :, 1::2] * (DEC[0] / DEC[1])).astype(np.float16)],
                  axis=2)
    in_maps = []
    for n in range(world):
        m = {"x": np.ascontiguousarray(xs[n])}
        m.update(consts)
        in_maps.append(m)

    res = run_bass_kernel_spmd(
        nc, in_maps, core_ids=list(range(world)),
        trace=bool(int(os.environ.get("WK_TRACE", "0"))))
    out = np.stack(
        [np.concatenate([r[f"out{i}"] for i in range(16)], axis=0).transpose(1, 0, 2)
         for r in res.results], axis=0).astype(np.float32)
    _CACHE["last_perf"] = res
    return out


# revision 65
# speedup vs baseline: 1.4254x; 1.4254x over previous
"""Trainium2 Bass kernel for nn_Center2D (DWT -> pool -> conv-BN-ReLU x2 -> deconv -> IDWT).

Self-contained: hardcodes shapes from the problem spec.
Sharding: pure data parallel, batch dim (8) across 8 cores; BN batch stats
synchronized with a tiny AllReduce (2x128 floats) per BN layer.

Layout strategy per core (one sample):
  io:    x is pre-split on host into even/odd w-columns and cast to f16
         (halves HBM read traffic and makes all front DVE reads
         contiguous); output is stored f16 and upcast on host.
  front: DWT-W as a 6-op ts/tt chain (stt runs at half DVE rate), scaled
         copies offloaded to the ACT engine, 1/D1*1/D2 fold absorbed into
         conv1 weights on host (positive, so max-pools commute); the
         DWT-H/pool wave trails one chunk so DVE never waits on ACT;
         conv1 matmuls issued per row-chunk overlap the front on the
         otherwise-idle PE.
  mid:   conv1 K-packed (ky=0,1 pairs -> 128-deep contraction, via a
         partition-shifted duplicate of the padded input), conv2 as 9
         K-packed matmuls, BN stats via accum_out during PSUM evacuation,
         tiny AllReduce per BN (warmed up by a dummy collective at t=0),
         BN+ReLU applied on DVE in halves so conv2/deconv start early.
  back:  deconv as 4 PE matmuls in h-halves, DRAM round-trip to put H on
         partitions (writes on the SWDGE queue, descriptor-bound
         transposed reads split 4-way over both HWDGE queues), PE matmul
         for IDWT-H with REC2 folded into the banded matrix, IDWT-W as
         ts/tt chains with the terminal odds op on ACT, f16 stores split
         across 16 ExternalOutput tensors.
"""

import os
import numpy as np

import concourse.bass as bass
import concourse.bacc as bacc
import concourse.tile as tile
from concourse import mybir
from concourse.bass_utils import run_bass_kernel_spmd

F32 = mybir.dt.float32
F16 = mybir.dt.float16
AF = mybir.ActivationFunctionType
ALU = mybir.AluOpType

REC = np.array([0.48296291314469025, 0.8365163037378079,
                0.22414386804185735, -0.12940952255092145], dtype=np.float64)
DEC = REC[::-1].copy()

N_CORES = int(os.environ.get("WK_CORES", "8"))
EPS = 1e-5

D0, D1, D2, D3 = (float(DEC[0]), float(DEC[1]), float(DEC[2]), float(DEC[3]))
# DWT-W chain carries 1/D1, DWT-H chain 1/D2 -> fold D1*D2 (positive, so
# the max-pools commute) into conv1 weights
FOLD = D1 * D2


# ---------------------------------------------------------------- host consts
def build_BH():
    """IDWT along one axis as a dense [128, 254] matrix, pre-scaled by REC2
    so the even-column IDWT-W tap needs no temporary."""
    B = np.zeros((128, 254), dtype=np.float64)
    for t in range(127):
        B[t,   2*t] += REC[2]
        B[t+1, 2*t] += REC[0]
        B[t,   2*t+1] += REC[3]
        B[t+1, 2*t+1] += REC[1]
    return (B * REC[2]).astype(np.float32)


def pack_consts(conv1_w, conv2_w, deconv_w, deconv_b,
                bn1_g, bn1_b, bn2_g, bn2_b):
    bhw = build_BH().astype(np.float16)          # [128, 254]

    w1 = conv1_w.astype(np.float64) * FOLD       # fold DWT chain scale
    # packed ky=0/1 pairs: rows 0:64 = ci(ky=0), 64:128 = ci(ky=1)
    w1p = np.zeros((128, 3 * 128), np.float16)
    w1s = np.zeros((64, 3 * 128), np.float16)    # ky=2
    for kx in range(3):
        w1p[0:64, kx*128:(kx+1)*128] = w1[:, :, 0, kx].T
        w1p[64:128, kx*128:(kx+1)*128] = w1[:, :, 1, kx].T
        w1s[:, kx*128:(kx+1)*128] = w1[:, :, 2, kx].T

    w2t = np.zeros((128, 9 * 128), np.float16)
    for ky in range(3):
        for kx in range(3):
            w2t[:, (ky*3+kx)*128:(ky*3+kx+1)*128] = conv2_w[:, :, ky, kx].T

    wdt = np.zeros((128, 4 * 64), np.float16)    # [ci, (k,l,o)]
    for k in range(2):
        for l in range(2):
            wdt[:, (k*2+l)*64:(k*2+l+1)*64] = deconv_w[:, :, k, l]

    return {
        "BHW": bhw,
        "w1p": w1p,
        "w1s": w1s,
        "w2t": w2t,
        "wdt": wdt,
        "db": deconv_b.reshape(64, 1).astype(np.float32),
        "bn1g": bn1_g.reshape(128, 1).astype(np.float32),
        "bn1b": bn1_b.reshape(128, 1).astype(np.float32),
        "bn2g": bn2_g.reshape(128, 1).astype(np.float32),
        "bn2b": bn2_b.reshape(128, 1).astype(np.float32),
    }


# ---------------------------------------------------------------- bass kernel
def build_nc(world=N_CORES, stage=None):
    if stage is None:
        stage = int(os.environ.get("WK_STAGE", "99"))
    nc = bacc.Bacc("TRN2", target_bir_lowering=False)
    use_cc = world > 1

    # x pre-split on host: [c, parity, h, 128] f16 (parity 0 = even w cols)
    x = nc.dram_tensor("x", (64, 2, 256, 128), F16, kind="ExternalInput")
    bhw_d = nc.dram_tensor("BHW", (128, 254), F16, kind="ExternalInput")
    w1p_d = nc.dram_tensor("w1p", (128, 384), F16, kind="ExternalInput")
    w1s_d = nc.dram_tensor("w1s", (64, 384), F16, kind="ExternalInput")
    w2t_d = nc.dram_tensor("w2t", (128, 1152), F16, kind="ExternalInput")
    wdt_d = nc.dram_tensor("wdt", (128, 256), F16, kind="ExternalInput")
    db_d = nc.dram_tensor("db", (64, 1), F32, kind="ExternalInput")
    bn_vecs = {n: nc.dram_tensor(n, (128, 1), F32, kind="ExternalInput")
               for n in ("bn1g", "bn1b", "bn2g", "bn2b")}
    # 16 separate f16 output tensors (h-slices); host upcasts to f32
    OUT_SPLITS = []
    h0 = 0
    for i in range(16):
        sz = 16 if (i % 8) != 7 else 15
        OUT_SPLITS.append((h0, sz))
        h0 += sz
    out_ds = [nc.dram_tensor(f"out{i}", (sz, 64, 254), F16, kind="ExternalOutput")
              for i, (_, sz) in enumerate(OUT_SPLITS)]

    scr2 = nc.dram_tensor("scr2", (64, 128, 128), F16, kind="Internal")
    cc_bufs = []
    for i in (0, 1, 2):
        cc_bufs.append((
            nc.dram_tensor(f"bn{i}_in", (128, 2), F32, kind="Internal"),
            nc.dram_tensor(f"bn{i}_out", (128, 2), F32, kind="Internal",
                           addr_space="Shared"),
        ))
    rg = [list(range(world))]
    cnt = float(world * 64 * 64)

    with tile.TileContext(nc) as tc, \
         tc.tile_pool(name="persist", bufs=1) as pp:
        def _body():
            # warmup collective: absorbs the ~11us first-call ncfw setup
            if use_cc:
                nc.gpsimd.collective_compute(
                    "AllReduce", ALU.add, replica_groups=rg,
                    ins=[cc_bufs[0][0][:]], outs=[cc_bufs[0][1][:]])

            # x viewed as [(g c) -> 128 partitions, h_local, w2]; partition
            # group g=0 holds h 0:128, g=1 holds h 128:256 of channel c
            xg = x[:].rearrange("c p (g r) w -> g p c (r w)", g=2)  # [2,2,64,128*128]

            # ---------- chunk 0 loads first (heads the dependency graph)
            xin_pool = tc.tile_pool(name="xin", bufs=2)
            xin = xin_pool.__enter__()
            xe0 = xin.tile([128, 32 * 128], F16, tag="xe")
            xo0 = xin.tile([128, 32 * 128], F16, tag="xo")
            # chunk 0 in 16-row pieces so the first DVE op starts sooner
            for c0 in (0, 2048):
                nc.sync.dma_start(xe0[0:64, c0:c0+2048],
                                  xg[0, 0, :, c0:c0+2048])
                nc.scalar.dma_start(xe0[64:128, c0:c0+2048],
                                    xg[1, 0, :, c0:c0+2048])
                nc.sync.dma_start(xo0[0:64, c0:c0+2048],
                                  xg[0, 1, :, c0:c0+2048])
                nc.scalar.dma_start(xo0[64:128, c0:c0+2048],
                                    xg[1, 1, :, c0:c0+2048])

            # conv1 input, with partition-shifted duplicate for K-packing:
            # parts 0:64 row r = padded row r; parts 64:128 row r = padded r+1
            in1b = pp.tile([128, 66 * 66], F16, name="in1b")
            nc.gpsimd.memset(in1b[:], 0.0)
            p1v = in1b[:].rearrange("p (r v) -> p r v", v=66)

            # ---------------- front: DWT + pool on DVE/GpSimd ----------
            front_pool = tc.tile_pool(name="front", bufs=1)
            fp = front_pool.__enter__()

            y_t = fp.tile([128, 128 * 128], F16, name="y_t")    # DWT-W out (y')
            y_v = y_t[:].rearrange("p (h t) -> p h t", t=128)
            y2_t = fp.tile([128, 64 * 128], F16, name="y2_t")   # DWT-H out (y2'')
            y2_v = y2_t[:].rearrange("p (s t) -> p s t", t=128)
            pw_t = fp.tile([128, 64 * 64], F16, name="pw_t")    # pool-W out
            pw_v = pw_t[:].rearrange("p (s u) -> p s u", u=64)

            # consts to SBUF (after chunk-0 loads in queue order; tiny ones
            # go on the gpsimd SWDGE queue so they don't delay the scalar
            # queue's first front ops)
            w1p_sb = pp.tile([128, 384], F16, name="w1p_sb")
            nc.sync.dma_start(w1p_sb[:], w1p_d[:])
            w1s_sb = pp.tile([64, 384], F16, name="w1s_sb")
            nc.gpsimd.dma_start(w1s_sb[:], w1s_d[:])
            bnv = {}
            for n, d in bn_vecs.items():
                t = pp.tile([128, 1], F32, name=f"{n}_sb")
                nc.gpsimd.dma_start(t[:], d[:])
                bnv[n] = t

            # conv1 state (PSUM banks live across the whole front)
            a1_sb = pp.tile([128, 4096], F16, name="a1_sb")
            junk = pp.tile([128, 512], F32, name="junk")
            s1b = pp.tile([128, 8], F32, name="s1b")
            s2b = pp.tile([128, 8], F32, name="s2b")
            a1v = a1_sb[:].rearrange("p (r q) -> p r q", q=64)
            psB_pool = tc.tile_pool(name="psB", bufs=8, space="PSUM")
            psB = psB_pool.__enter__()
            ps_list = [psB.tile([128, 512], F32, tag="psB", name=f"c1ps{i}")
                       for i in range(8)]

            def conv1_chunk(ch):
                q0 = ch * 8
                for kx in range(3):
                    nc.tensor.matmul(ps_list[ch][:],
                                     w1s_sb[:, kx*128:(kx+1)*128],
                                     p1v[0:64, q0+2:q0+10, kx:kx+64],
                                     start=(kx == 0), stop=False)
                for kx in range(3):
                    nc.tensor.matmul(ps_list[ch][:],
                                     w1p_sb[:, kx*128:(kx+1)*128],
                                     p1v[:, q0:q0+8, kx:kx+64],
                                     start=False, stop=(kx == 2))
                # evacuate + BN1 stats (conv bias is a no-op before BN)
                nc.vector.tensor_scalar(a1v[:, q0:q0+8, :], ps_list[ch][:],
                                        1.0, 0.0, ALU.mult, ALU.add,
                                        accum_out=s1b[:, ch:ch+1])
                nc.scalar.activation(junk[:], ps_list[ch][:], AF.Square,
                                     accum_out=s2b[:, ch:ch+1])

            r30, r10, r02 = D3 / D2, D1 / D0, D0 / D2
            m0, m1 = (D1 + D2) / D2, (D0 + D3) / D2
            # DWT-W ts/tt chain ratios (stt runs at half DVE rate, so cheap
            # ops beat 3 stt): y'' = y/D1. The host pre-scales the O plane
            # by D0/D1, so u2 needs no trailing scale.
            w_s1, w_s2 = D3 / D1, D2 / D0
            mw0 = (D1 + D2) / D1
            mw1 = ((D0 + D3) / D1) * (D1 / D0)   # for the D0/D1-scaled O

            def dwt_h_piece(a, b, v1_p, v2_p):
                """y2''[s] for s_local in [a, b) (a >= 1), both part groups.
                v1 = (d3/d2) y'[2s-2] + y'[2s-1]; v2 = (d1/d0) y'[2s] + y'[2s+1]
                y2'' = (d0/d2) v2 + v1; scaled copies on ACT, adds on DVE"""
                n = b - a
                v1 = v1_p[:].rearrange("p (s t) -> p s t", t=128)[:, 0:n, :]
                v2 = v2_p[:].rearrange("p (s t) -> p s t", t=128)[:, 0:n, :]
                nc.vector.scalar_tensor_tensor(v1, y_v[:, 2*a-2:2*b-3:2, :],
                                               r30, y_v[:, 2*a-1:2*b-2:2, :],
                                               ALU.mult, ALU.add)
                nc.vector.scalar_tensor_tensor(v2, y_v[:, 2*a:2*b-1:2, :],
                                               r10, y_v[:, 2*a+1:2*b:2, :],
                                               ALU.mult, ALU.add)
                nc.vector.scalar_tensor_tensor(y2_v[:, a:b, :], v2, r02, v1,
                                               ALU.mult, ALU.add)

            conv1_done = set()

            def issue_conv1(chs):
                for ch in chs:
                    if ch not in conv1_done:
                        conv1_done.add(ch)
                        conv1_chunk(ch)

            with tc.tile_pool(name="twp", bufs=2) as twp, \
                 tc.tile_pool(name="thp", bufs=2) as thp:

                def wave(hc):
                    # DWT-H + pools + assembly for the s-range chunk hc
                    # completed; issued one chunk late so these DVE ops run
                    # while ACT produces the next chunk's scaled copies
                    a = max(1, 16 * hc)
                    b = 16 * hc + 16
                    v1_p = thp.tile([128, 16 * 128], F16, tag="v1")
                    v2_p = thp.tile([128, 16 * 128], F16, tag="v2")
                    dwt_h_piece(a, b, v1_p, v2_p)
                    if hc == 0:
                        # s_local=0, group 0 mirror: m0 y'[0] + m1 y'[1]
                        nc.vector.scalar_tensor_tensor(
                            y2_v[0:64, 0:1, :], y_v[0:64, 0:1, :], m0 / m1,
                            y_v[0:64, 1:2, :], ALU.mult, ALU.add)
                        nc.vector.tensor_scalar(y2_v[0:64, 0:1, :],
                                                y2_v[0:64, 0:1, :],
                                                m1, None, ALU.mult)
                        nc.vector.tensor_tensor(pw_v[0:64, 0:1, :],
                                                y2_v[0:64, 0:1, 0::2],
                                                y2_v[0:64, 0:1, 1::2], ALU.max)
                    # pool-W for those s rows (on the otherwise-idle Pool eng)
                    nc.vector.tensor_tensor(pw_v[:, a:b, :],
                                            y2_v[:, a:b, 0::2],
                                            y2_v[:, a:b, 1::2], ALU.max)
                    if hc == 3:
                        # s_local=0 group 1 seam: needs y' rows 126,127 of g0
                        seam = fp.tile([128, 2 * 128], F16, name="seam")
                        nc.sync.dma_start(seam[64:128, :],
                                          y_t[0:64, 126*128:128*128])
                        seam_v = seam[:].rearrange("p (h t) -> p h t", t=128)
                        sv1 = fp.tile([128, 128], F16, name="sv1")
                        sv2 = fp.tile([128, 128], F16, name="sv2")
                        nc.vector.scalar_tensor_tensor(
                            sv1[64:128, :].rearrange("p (a t) -> p a t", a=1),
                            seam_v[64:128, 0:1, :], r30,
                            seam_v[64:128, 1:2, :], ALU.mult, ALU.add)
                        nc.vector.scalar_tensor_tensor(
                            sv2[64:128, :].rearrange("p (a t) -> p a t", a=1),
                            y_v[64:128, 0:1, :], r10,
                            y_v[64:128, 1:2, :], ALU.mult, ALU.add)
                        nc.vector.scalar_tensor_tensor(
                            y2_v[64:128, 0:1, :],
                            sv2[64:128, :].rearrange("p (a t) -> p a t", a=1),
                            r02,
                            sv1[64:128, :].rearrange("p (a t) -> p a t", a=1),
                            ALU.mult, ALU.add)
                        nc.vector.tensor_tensor(pw_v[64:128, 0:1, :],
                                                y2_v[64:128, 0:1, 0::2],
                                                y2_v[64:128, 0:1, 1::2],
                                                ALU.max)
                        # deferred pooled row q=32 (block1 row 32) + its
                        # cross-partition copy into block0 row 33
                        nc.vector.tensor_tensor(p1v[64:128, 32:33, 1:65],
                                                pw_v[64:128, 0:1, :],
                                                pw_v[64:128, 1:2, :], ALU.max)
                        nc.sync.dma_start(p1v[0:64, 33:34, :],
                                          p1v[64:128, 32:33, :])
                    # pool-H for pooled rows this chunk completes:
                    # g0: q rows 8hc..8hc+7 -> p1 block0 rows 1+8hc..8+8hc
                    # g1: q rows 32+8hc..39+8hc -> p1 block1 rows 32+8hc..
                    # (g1 pooled row 32 needs the hc==3 seam -> deferred,
                    #  so at hc==0 only rows 33..39 are written here)
                    qa = 8 * hc
                    src0 = pw_v[0:64, 2*qa:2*qa+16, :]
                    nc.vector.tensor_tensor(p1v[0:64, 1+qa:9+qa, 1:65],
                                            src0[:, 0::2, :], src0[:, 1::2, :],
                                            ALU.max)
                    r1a = 33 if hc == 0 else 32 + qa    # block1 first row
                    src1 = pw_v[64:128, 2*(r1a-32):2*qa+16, :]
                    nc.vector.tensor_tensor(p1v[64:128, r1a:40+qa, 1:65],
                                            src1[:, 0::2, :], src1[:, 1::2, :],
                                            ALU.max)
                    # cross-partition copies for the packed layout
                    # block0 rows r1a+1..40+qa <- block1 rows r1a..39+qa
                    nc.sync.dma_start(p1v[0:64, r1a+1:41+qa, :],
                                      p1v[64:128, r1a:40+qa, :])
                    # block1 rows qa..qa+7 <- block0 rows 1+qa..8+qa
                    nc.scalar.dma_start(p1v[64:128, qa:qa+8, :],
                                        p1v[0:64, 1+qa:9+qa, :])
                    # conv1 chunks that just became ready
                    if stage > 2:
                        if hc == 1:
                            issue_conv1([0])
                        elif hc == 2:
                            issue_conv1([1, 5])

                tiles = {0: (xe0, xo0)}

                def load_chunk(h):
                    xe = xin.tile([128, 32 * 128], F16, tag="xe")
                    xo = xin.tile([128, 32 * 128], F16, tag="xo")
                    o0 = h * 4096
                    nc.sync.dma_start(xe[0:64], xg[0, 0, :, o0:o0+4096])
                    nc.scalar.dma_start(xe[64:128], xg[1, 0, :, o0:o0+4096])
                    nc.sync.dma_start(xo[0:64], xg[0, 1, :, o0:o0+4096])
                    nc.scalar.dma_start(xo[64:128], xg[1, 1, :, o0:o0+4096])
                    tiles[h] = (xe, xo)

                for hc in range(4):          # h-chunks of 32 rows
                    # prefetch the NEXT chunk now: dispatched ahead of this
                    # iteration's ACT ops, its data lands before the next
                    # iteration's scaled copies need it
                    if hc < 3:
                        load_chunk(hc + 1)
                    # previous chunk's DWT-H/pool wave first: its DVE ops are
                    # ready now and fill the DVE while ACT produces this
                    # chunk's scaled copies
                    if hc >= 1:
                        wave(hc - 1)
                    xe, xo = tiles.pop(hc)
                    ev = xe[:].rearrange("p (h w) -> p h w", w=128)
                    ov = xo[:].rearrange("p (h w) -> p h w", w=128)
                    yc = y_v[:, hc*32:(hc+1)*32, :]
                    # DWT-W 5-op chain (y'' = y/D1, O plane pre-scaled by
                    # D0/D1 on host): u1 = (D3/D1) E' + E''
                    # u2 = (D2/D0) Os' + Os'';  y'' = u1 + u2
                    # Scaled copies go to the idle ACT engine (except chunk
                    # 0, where ACT would delay the DVE start); chunk 0 runs
                    # in 16-row sub-chunks to shorten the lead-in.
                    for r0, rn in (((0, 16), (16, 16)) if hc == 0
                                   else ((0, 32),)):
                        evr = ev[:, r0:r0+rn, :]
                        ovr = ov[:, r0:r0+rn, :]
                        ycr = yc[:, r0:r0+rn, :]
                        u1 = twp.tile([128, 32 * 127], F16, tag="u1")
                        u2 = twp.tile([128, 32 * 127], F16, tag="u2")
                        u1v = u1[:].rearrange("p (h t) -> p h t",
                                              t=127)[:, 0:rn, :]
                        u2v = u2[:].rearrange("p (h t) -> p h t",
                                              t=127)[:, 0:rn, :]
                        if hc == 0:
                            nc.vector.tensor_scalar(u1v, evr[:, :, 0:127],
                                                    w_s1, None, ALU.mult)
                            nc.vector.tensor_scalar(u2v, ovr[:, :, 0:127],
                                                    w_s2, None, ALU.mult)
                        else:
                            nc.scalar.activation(u1v, evr[:, :, 0:127],
                                                 AF.Identity, scale=w_s1)
                            nc.scalar.activation(u2v, ovr[:, :, 0:127],
                                                 AF.Identity, scale=w_s2)
                        nc.vector.tensor_tensor(u1v, u1v, evr[:, :, 1:128],
                                                ALU.add)
                        nc.vector.tensor_tensor(u2v, u2v, ovr[:, :, 1:128],
                                                ALU.add)
                        nc.vector.tensor_tensor(ycr[:, :, 1:128], u1v, u2v,
                                                ALU.add)
                        # t=0 mirror: y''[0] = mw0 E[0] + mw1 Os[0]
                        nc.vector.scalar_tensor_tensor(
                            ycr[:, :, 0:1],
                            evr[:, :, 0:1], mw0 / mw1, ovr[:, :, 0:1],
                            ALU.mult, ALU.add)
                        nc.vector.tensor_scalar(ycr[:, :, 0:1],
                                                ycr[:, :, 0:1],
                                                mw1, None, ALU.mult)
                wave(3)
                # mid/back consts (queues free once front loads are done)
                w2t_sb = pp.tile([128, 1152], F16, name="w2t_sb")
                nc.sync.dma_start(w2t_sb[:], w2t_d[:])
                wdt_sb = pp.tile([128, 256], F16, name="wdt_sb")
                nc.scalar.dma_start(wdt_sb[:], wdt_d[:])
                db_sb = pp.tile([64, 1], F32, name="db_sb")
                nc.sync.dma_start(db_sb[:], db_d[:])
                bhw_sb = pp.tile([128, 254], F16, name="bhw_sb")
                nc.scalar.dma_start(bhw_sb[:], bhw_d[:])

            front_pool.__exit__(None, None, None)
            xin_pool.__exit__(None, None, None)
            if stage <= 2:
                psB_pool.__exit__(None, None, None)
                return

            # ---------------- conv1 rest (+BN1) ----------------
            mid_pool = tc.tile_pool(name="mid", bufs=1)
            mp = mid_pool.__enter__()
            in2_pad = mp.tile([128, 66 * 66], F16, name="in2_pad")
            nc.gpsimd.memset(in2_pad[:], 0.0)
            issue_conv1([2, 3, 4, 6, 7])
            psB_pool.__exit__(None, None, None)

            if stage <= 3:
                mid_pool.__exit__(None, None, None)
                return
            sc1, bi1 = _bn_coeffs(nc, pp, s1b, s2b, cc_bufs[1], rg, cnt,
                                  bnv["bn1g"], bnv["bn1b"], use_cc, tag=1)

            # BN1 + ReLU on DVE (faster than ACT, and DVE is idle here),
            # split in h-halves so conv2 chunks can start early
            p2v = in2_pad[:].rearrange("p (r v) -> p r v", v=66)

            def bn1_apply(hh):
                dst = p2v[:, 1+32*hh:33+32*hh, 1:65]
                nc.vector.tensor_scalar(dst, a1v[:, 32*hh:32*hh+32, :],
                                        sc1[:], bi1[:], ALU.mult, ALU.add)
                nc.vector.tensor_scalar_max(dst, dst, 0.0)

            if stage <= 4:
                bn1_apply(0)
                bn1_apply(1)
                mid_pool.__exit__(None, None, None)
                return
            # ---------------- conv2 (+BN2 stats) ----------------
            h2_sb = mp.tile([128, 4096], F16, name="h2_sb")
            h2v = h2_sb[:].rearrange("p (r q) -> p r q", q=64)
            s1c = pp.tile([128, 8], F32, name="s1c")
            s2c = pp.tile([128, 8], F32, name="s2c")
            with tc.tile_pool(name="psC", bufs=8, space="PSUM") as psC:
                ps_list2 = [psC.tile([128, 512], F32, tag="psC", name=f"c2ps{i}")
                            for i in range(8)]

                def conv2_chunk(ch):
                    p0 = ch * 8
                    for ti in range(9):
                        ky, kx = divmod(ti, 3)
                        rhs = p2v[:, p0+ky:p0+ky+8, kx:kx+64]
                        nc.tensor.matmul(ps_list2[ch][:],
                                         w2t_sb[:, ti*128:(ti+1)*128],
                                         rhs, start=(ti == 0), stop=(ti == 8))
                    nc.vector.tensor_scalar(h2v[:, p0:p0+8, :],
                                            ps_list2[ch][:],
                                            1.0, 0.0, ALU.mult, ALU.add,
                                            accum_out=s1c[:, ch:ch+1])
                    nc.scalar.activation(junk[:], ps_list2[ch][:], AF.Square,
                                         accum_out=s2c[:, ch:ch+1])

                bn1_apply(0)
                for ch in (0, 1, 2):
                    conv2_chunk(ch)
                bn1_apply(1)
                for ch in (3, 4, 5, 6, 7):
                    conv2_chunk(ch)

            sc2, bi2 = _bn_coeffs(nc, pp, s1c, s2c, cc_bufs[2], rg, cnt,
                                  bnv["bn2g"], bnv["bn2b"], use_cc, tag=2)

            def bn2_apply(r0, rn):
                dst = h2v[:, r0:r0+rn, :]
                nc.vector.tensor_scalar(dst, dst, sc2[:], bi2[:],
                                        ALU.mult, ALU.add)
                nc.vector.tensor_scalar_max(dst, dst, 0.0)

            if stage <= 5:
                bn2_apply(0, 64)
                mid_pool.__exit__(None, None, None)
                return
            # ---------------- deconv ----------------
            dth = pp.tile([128, 64 * 128], F16, name="dth")
            dth_v = dth[:].rearrange("p (o w) -> p o w", w=128)
            scr2_h = scr2[:].rearrange("o h w -> h o w")
            d_sb = mp.tile([64, 128 * 128], F16, name="d_sb")
            dv = d_sb[:].rearrange("p (h w) -> p h w", w=128)
            with tc.tile_pool(name="psD", bufs=8, space="PSUM") as psD:
                # scr2 writes ride the SWDGE queue; the descriptor-bound
                # transposed reads get both HWDGE queues, split 4-way so all
                # 16 SDMA engines stay fed
                for r0, rn in ((0, 32), (32, 32)):
                    bn2_apply(r0, rn)
                    for kl in range(4):
                        k, l = divmod(kl, 2)
                        for ch in range(rn // 8):
                            p0 = r0 + ch * 8
                            ps = psD.tile([64, 512], F32, tag="psD")
                            nc.tensor.matmul(ps[:], wdt_sb[:, kl*64:(kl+1)*64],
                                             h2v[:, p0:p0+8, :],
                                             start=True, stop=True)
                            dst = dv[:, 2*p0+k:2*p0+k+15:2, l::2]
                            if (kl * 4 + ch) % 2 == 0:
                                nc.vector.tensor_scalar(dst, ps[:], 1.0,
                                                        db_sb[:],
                                                        ALU.mult, ALU.add)
                            else:
                                nc.scalar.activation(dst, ps[:], AF.Identity,
                                                     bias=db_sb[:], scale=1.0)
                    h0, hn = 2 * r0, 2 * rn
                    nc.gpsimd.dma_start(scr2[:, h0:h0+hn, :],
                                        dv[:, h0:h0+hn, :])
                    for oq in range(4):
                        eng = nc.sync if oq % 2 == 0 else nc.scalar
                        eng.dma_start(dth_v[h0:h0+hn, oq*16:(oq+1)*16, :],
                                      scr2_h[h0:h0+hn, oq*16:(oq+1)*16, :])
            mid_pool.__exit__(None, None, None)
            if stage <= 6:
                return

            # ---------------- IDWT-H on PE, IDWT-W spread ----------------
            # g2 = REC2 * (IDWT-H of dth)  (REC2 folded into BHW)
            # out evens: o[2t] = g2[t] + (REC0/REC2) g2[t+1]   (one stt)
            # out odds:  o[2t+1] = (REC3/REC2) g2[t] + (REC1/REC2) g2[t+1]
            q0r, q1r, q3r = REC[0] / REC[2], REC[1] / REC[2], REC[3] / REC[2]
            with tc.tile_pool(name="psE", bufs=8, space="PSUM") as psE, \
                 tc.tile_pool(name="gpool", bufs=2) as gpool, \
                 tc.tile_pool(name="twpool", bufs=2) as twpool, \
                 tc.tile_pool(name="opool", bufs=3) as opool:
                for blk in range(2):
                    g_t = gpool.tile([127, 8192], F16, tag="g")
                    g_v = g_t[:].rearrange("p (o w) -> p o w", w=128)
                    for nch in range(16):
                        ps = psE.tile([127, 512], F32, tag="psE")
                        nc.tensor.matmul(ps[:], bhw_sb[:, blk*127:blk*127+127],
                                         dth[:, nch*512:(nch+1)*512],
                                         start=True, stop=True)
                        dst = g_t[:, nch*512:(nch+1)*512]
                        if nch % 2 == 0:
                            nc.vector.tensor_copy(dst, ps[:])
                        else:
                            nc.scalar.copy(dst, ps[:])
                    # blk1 split finer so the final store tail is short
                    pieces = ((0, 32), (32, 32)) if blk == 0 else \
                             ((0, 32), (32, 16), (48, 16))
                    for pi, (r0, rn) in enumerate(pieces):
                        gh = g_v[:, r0:r0+rn, :]
                        o_t = opool.tile([127, 32 * 254], F16, tag="o")
                        o_v = o_t[:].rearrange("p (o w) -> p o w",
                                               w=254)[:, 0:rn, :]
                        # ts/tt chains (stt runs at half DVE rate); the
                        # terminal odds op goes to ACT so nothing on the DVE
                        # critical path waits for the slower engine
                        # evens: o[2t] = q0r g2[t+1] + g2[t]
                        ae = twpool.tile([127, 32 * 127], F16, tag="ae")
                        aev = ae[:].rearrange("p (o w) -> p o w",
                                              w=127)[:, 0:rn, :]
                        nc.vector.tensor_scalar(aev, gh[:, :, 1:128], q0r,
                                                None, ALU.mult)
                        nc.vector.tensor_tensor(o_v[:, :, 0:253:2], aev,
                                                gh[:, :, 0:127], ALU.add)
                        # odds: o[2t+1] = q3r ((q1r/q3r) g2[t+1] + g2[t])
                        ao = twpool.tile([127, 32 * 127], F16, tag="ao")
                        aov = ao[:].rearrange("p (o w) -> p o w",
                                              w=127)[:, 0:rn, :]
                        nc.vector.tensor_scalar(aov, gh[:, :, 1:128], q1r/q3r,
                                                None, ALU.mult)
                        nc.vector.tensor_tensor(aov, aov, gh[:, :, 0:127],
                                                ALU.add)
                        nc.scalar.activation(o_v[:, :, 1:254:2], aov,
                                             AF.Identity, scale=q3r)
                        for i in range(8):
                            oi = blk * 8 + i
                            h0, sz = OUT_SPLITS[oi]
                            p0 = h0 - blk * 127
                            eng = nc.sync if (i + pi) % 2 == 0 else nc.scalar
                            eng.dma_start(out_ds[oi][:, r0:r0+rn, :],
                                          o_v[p0:p0+sz])

        _body()
    nc.compile()
    return nc


def _bn_coeffs(nc, pp, s1b, s2b, cc_pair, rg, cnt, g_sb, b_sb, use_cc, tag):
    """Reduce per-chunk sums, AllReduce across cores, return (scale, bias) [128,1]."""
    ALU = mybir.AluOpType
    sl = pp.tile([128, 2], F32, name=f"bn{tag}_sl")
    nc.vector.tensor_reduce(sl[:, 0:1], s1b[:], mybir.AxisListType.X, ALU.add)
    nc.vector.tensor_reduce(sl[:, 1:2], s2b[:], mybir.AxisListType.X, ALU.add)
    cc_in, cc_out = cc_pair
    sg = pp.tile([128, 2], F32, name=f"bn{tag}_sg")
    if use_cc:
        nc.sync.dma_start(cc_in[:], sl[:])
        nc.gpsimd.collective_compute(
            "AllReduce", ALU.add, replica_groups=rg,
            ins=[cc_in[:]], outs=[cc_out[:]])
        nc.sync.dma_start(sg[:], cc_out[:])
    else:
        nc.vector.tensor_copy(sg[:], sl[:])

    m = pp.tile([128, 1], F32, name=f"bn{tag}_m")
    vpe = pp.tile([128, 1], F32, name=f"bn{tag}_v")
    t0 = pp.tile([128, 1], F32, name=f"bn{tag}_t0")
    nc.vector.tensor_scalar(m[:], sg[:, 0:1], 1.0 / cnt, None, ALU.mult)
    nc.vector.tensor_tensor(t0[:], m[:], m[:], ALU.mult)          # m^2
    nc.vector.tensor_scalar(vpe[:], sg[:, 1:2], 1.0 / cnt, float(EPS), ALU.mult,
                            ALU.add)                              # E[x^2]+eps
    nc.vector.tensor_tensor(vpe[:], vpe[:], t0[:], ALU.subtract)  # var+eps
    # rsqrt with one Newton step (ACT Sqrt is low-precision)
    s0 = pp.tile([128, 1], F32, name=f"bn{tag}_s0")
    y0 = pp.tile([128, 1], F32, name=f"bn{tag}_y0")
    nc.scalar.activation(s0[:], vpe[:], mybir.ActivationFunctionType.Sqrt)
    nc.vector.reciprocal(y0[:], s0[:])
    t1 = pp.tile([128, 1], F32, name=f"bn{tag}_t1")
    nc.vector.tensor_tensor(t1[:], y0[:], y0[:], ALU.mult)
    nc.vector.tensor_tensor(t1[:], t1[:], vpe[:], ALU.mult)
    nc.vector.tensor_scalar(t1[:], t1[:], -0.5, 1.5, ALU.mult, ALU.add)
    nc.vector.tensor_tensor(y0[:], y0[:], t1[:], ALU.mult)        # refined rsqrt
    sc = pp.tile([128, 1], F32, name=f"bn{tag}_sc")
    bi = pp.tile([128, 1], F32, name=f"bn{tag}_bi")
    nc.vector.tensor_tensor(sc[:], y0[:], g_sb[:], ALU.mult)
    nc.vector.tensor_tensor(t0[:], m[:], sc[:], ALU.mult)
    nc.vector.tensor_tensor(bi[:], b_sb[:], t0[:], ALU.subtract)
    return sc, bi


# ---------------------------------------------------------------- entry point
_CACHE = {}


def kernel(x, conv1_w, conv1_b, bn1_g, bn1_b, conv2_w, conv2_b, bn2_g, bn2_b,
           deconv_w, deconv_b):
    world = N_CORES
    if "nc" not in _CACHE:
        _CACHE["nc"] = build_nc(world)
    nc = _CACHE["nc"]

    consts = pack_consts(np.asarray(conv1_w), np.asarray(conv2_w),
                         np.asarray(deconv_w), np.asarray(deconv_b),
                         np.asarray(bn1_g), np.asarray(bn1_b),
                         np.asarray(bn2_g), np.asarray(bn2_b))
    x = np.asarray(x)
    # host-side prep: f16 + even/odd w split -> [c, parity, h, 128];
    # the odd plane carries the D0/D1 factor of the DWT-W chain
    xs = np.stack([x[:, :, :, 0::2].astype(np.float16),
                   (x[:, :, :, 1::2] * (DEC[0] / DEC[1])).astype(np.float16)],
                  axis=2)
    in_maps = []
    for n in range(world):
        m = {"x": np.ascontiguousarray(xs[n])}
        m.update(consts)
        in_maps.append(m)

    res = run_bass_kernel_spmd(
        nc, in_maps, core_ids=list(range(world)),
        trace=bool(int(os.environ.get("WK_TRACE", "0"))))
    out = np.stack(
        [np.concatenate([r[f"out{i}"] for i in range(16)], axis=0).transpose(1, 0, 2)
         for r in res.results], axis=0).astype(np.float32)
    _CACHE["last_perf"] = res
    return out


# revision 66
# speedup vs baseline: 1.4300x; 1.0033x over previous
"""Trainium2 Bass kernel for nn_Center2D (DWT -> pool -> conv-BN-ReLU x2 -> deconv -> IDWT).

Self-contained: hardcodes shapes from the problem spec.
Sharding: pure data parallel, batch dim (8) across 8 cores; BN batch stats
synchronized with a tiny AllReduce (2x128 floats) per BN layer.

Layout strategy per core (one sample):
  io:    x is pre-split on host into even/odd w-columns and cast to f16
         (halves HBM read traffic and makes all front DVE reads
         contiguous); output is stored f16 and upcast on host.
  front: DWT-W as a 6-op ts/tt chain (stt runs at half DVE rate), scaled
         copies offloaded to the ACT engine, 1/D1*1/D2 fold absorbed into
         conv1 weights on host (positive, so max-pools commute); the
         DWT-H/pool wave trails one chunk so DVE never waits on ACT;
         conv1 matmuls issued per row-chunk overlap the front on the
         otherwise-idle PE.
  mid:   conv1 K-packed (ky=0,1 pairs -> 128-deep contraction, via a
         partition-shifted duplicate of the padded input), conv2 as 9
         K-packed matmuls, BN stats via accum_out during PSUM evacuation,
         tiny AllReduce per BN (warmed up by a dummy collective at t=0),
         BN+ReLU applied on DVE in halves so conv2/deconv start early.
  back:  deconv as 4 PE matmuls in h-halves, DRAM round-trip to put H on
         partitions (writes on the SWDGE queue, descriptor-bound
         transposed reads split 4-way over both HWDGE queues), PE matmul
         for IDWT-H with REC2 folded into the banded matrix, IDWT-W as
         ts/tt chains with the terminal odds op on ACT, f16 stores split
         across 16 ExternalOutput tensors.
"""

import os
import numpy as np

import concourse.bass as bass
import concourse.bacc as bacc
import concourse.tile as tile
from concourse import mybir
from concourse.bass_utils import run_bass_kernel_spmd

F32 = mybir.dt.float32
F16 = mybir.dt.float16
AF = mybir.ActivationFunctionType
ALU = mybir.AluOpType

REC = np.array([0.48296291314469025, 0.8365163037378079,
                0.22414386804185735, -0.12940952255092145], dtype=np.float64)
DEC = REC[::-1].copy()

N_CORES = int(os.environ.get("WK_CORES", "8"))
EPS = 1e-5

D0, D1, D2, D3 = (float(DEC[0]), float(DEC[1]), float(DEC[2]), float(DEC[3]))
# DWT-W chain carries 1/D1, DWT-H chain 1/D2 -> fold D1*D2 (positive, so
# the max-pools commute) into conv1 weights
FOLD = D1 * D2


# ---------------------------------------------------------------- host consts
def build_BH():
    """IDWT along one axis as a dense [128, 254] matrix, pre-scaled by REC2
    so the even-column IDWT-W tap needs no temporary."""
    B = np.zeros((128, 254), dtype=np.float64)
    for t in range(127):
        B[t,   2*t] += REC[2]
        B[t+1, 2*t] += REC[0]
        B[t,   2*t+1] += REC[3]
        B[t+1, 2*t+1] += REC[1]
    return (B * REC[2]).astype(np.float32)


def pack_consts(conv1_w, conv2_w, deconv_w, deconv_b,
                bn1_g, bn1_b, bn2_g, bn2_b):
    bhw = build_BH().astype(np.float16)          # [128, 254]

    w1 = conv1_w.astype(np.float64) * FOLD       # fold DWT chain scale
    # packed ky=0/1 pairs: rows 0:64 = ci(ky=0), 64:128 = ci(ky=1)
    w1p = np.zeros((128, 3 * 128), np.float16)
    w1s = np.zeros((64, 3 * 128), np.float16)    # ky=2
    for kx in range(3):
        w1p[0:64, kx*128:(kx+1)*128] = w1[:, :, 0, kx].T
        w1p[64:128, kx*128:(kx+1)*128] = w1[:, :, 1, kx].T
        w1s[:, kx*128:(kx+1)*128] = w1[:, :, 2, kx].T

    w2t = np.zeros((128, 9 * 128), np.float16)
    for ky in range(3):
        for kx in range(3):
            w2t[:, (ky*3+kx)*128:(ky*3+kx+1)*128] = conv2_w[:, :, ky, kx].T

    wdt = np.zeros((128, 4 * 64), np.float16)    # [ci, (k,l,o)]
    for k in range(2):
        for l in range(2):
            wdt[:, (k*2+l)*64:(k*2+l+1)*64] = deconv_w[:, :, k, l]

    return {
        "BHW": bhw,
        "w1p": w1p,
        "w1s": w1s,
        "w2t": w2t,
        "wdt": wdt,
        "db": deconv_b.reshape(64, 1).astype(np.float32),
        "bn1g": bn1_g.reshape(128, 1).astype(np.float32),
        "bn1b": bn1_b.reshape(128, 1).astype(np.float32),
        "bn2g": bn2_g.reshape(128, 1).astype(np.float32),
        "bn2b": bn2_b.reshape(128, 1).astype(np.float32),
    }


# ---------------------------------------------------------------- bass kernel
def build_nc(world=N_CORES, stage=None):
    if stage is None:
        stage = int(os.environ.get("WK_STAGE", "99"))
    nc = bacc.Bacc("TRN2", target_bir_lowering=False)
    use_cc = world > 1

    # x pre-split on host: [c, parity, h, 128] f16 (parity 0 = even w cols)
    x = nc.dram_tensor("x", (64, 2, 256, 128), F16, kind="ExternalInput")
    bhw_d = nc.dram_tensor("BHW", (128, 254), F16, kind="ExternalInput")
    w1p_d = nc.dram_tensor("w1p", (128, 384), F16, kind="ExternalInput")
    w1s_d = nc.dram_tensor("w1s", (64, 384), F16, kind="ExternalInput")
    w2t_d = nc.dram_tensor("w2t", (128, 1152), F16, kind="ExternalInput")
    wdt_d = nc.dram_tensor("wdt", (128, 256), F16, kind="ExternalInput")
    db_d = nc.dram_tensor("db", (64, 1), F32, kind="ExternalInput")
    bn_vecs = {n: nc.dram_tensor(n, (128, 1), F32, kind="ExternalInput")
               for n in ("bn1g", "bn1b", "bn2g", "bn2b")}
    # 16 separate f16 output tensors (h-slices); host upcasts to f32
    OUT_SPLITS = []
    h0 = 0
    for i in range(16):
        sz = 16 if (i % 8) != 7 else 15
        OUT_SPLITS.append((h0, sz))
        h0 += sz
    out_ds = [nc.dram_tensor(f"out{i}", (sz, 64, 254), F16, kind="ExternalOutput")
              for i, (_, sz) in enumerate(OUT_SPLITS)]

    scr2 = nc.dram_tensor("scr2", (64, 128, 128), F16, kind="Internal")
    cc_bufs = []
    for i in (0, 1, 2):
        cc_bufs.append((
            nc.dram_tensor(f"bn{i}_in", (128, 2), F32, kind="Internal"),
            nc.dram_tensor(f"bn{i}_out", (128, 2), F32, kind="Internal",
                           addr_space="Shared"),
        ))
    rg = [list(range(world))]
    cnt = float(world * 64 * 64)

    with tile.TileContext(nc) as tc, \
         tc.tile_pool(name="persist", bufs=1) as pp:
        def _body():
            # warmup collective: absorbs the ~11us first-call ncfw setup
            if use_cc:
                nc.gpsimd.collective_compute(
                    "AllReduce", ALU.add, replica_groups=rg,
                    ins=[cc_bufs[0][0][:]], outs=[cc_bufs[0][1][:]])

            # x viewed as [(g c) -> 128 partitions, h_local, w2]; partition
            # group g=0 holds h 0:128, g=1 holds h 128:256 of channel c
            xg = x[:].rearrange("c p (g r) w -> g p c (r w)", g=2)  # [2,2,64,128*128]

            # ---------- chunk 0 loads first (heads the dependency graph)
            xin_pool = tc.tile_pool(name="xin", bufs=3)
            xin = xin_pool.__enter__()
            xe0 = xin.tile([128, 32 * 128], F16, tag="xe")
            xo0 = xin.tile([128, 32 * 128], F16, tag="xo")
            # chunk 0 in 16-row pieces so the first DVE op starts sooner
            for c0 in (0, 2048):
                nc.sync.dma_start(xe0[0:64, c0:c0+2048],
                                  xg[0, 0, :, c0:c0+2048])
                nc.scalar.dma_start(xe0[64:128, c0:c0+2048],
                                    xg[1, 0, :, c0:c0+2048])
                nc.sync.dma_start(xo0[0:64, c0:c0+2048],
                                  xg[0, 1, :, c0:c0+2048])
                nc.scalar.dma_start(xo0[64:128, c0:c0+2048],
                                    xg[1, 1, :, c0:c0+2048])

            # conv1 input, with partition-shifted duplicate for K-packing:
            # parts 0:64 row r = padded row r; parts 64:128 row r = padded r+1
            in1b = pp.tile([128, 66 * 66], F16, name="in1b")
            nc.gpsimd.memset(in1b[:], 0.0)
            p1v = in1b[:].rearrange("p (r v) -> p r v", v=66)

            # ---------------- front: DWT + pool on DVE/GpSimd ----------
            front_pool = tc.tile_pool(name="front", bufs=1)
            fp = front_pool.__enter__()

            y_t = fp.tile([128, 128 * 128], F16, name="y_t")    # DWT-W out (y')
            y_v = y_t[:].rearrange("p (h t) -> p h t", t=128)
            y2_t = fp.tile([128, 64 * 128], F16, name="y2_t")   # DWT-H out (y2'')
            y2_v = y2_t[:].rearrange("p (s t) -> p s t", t=128)
            pw_t = fp.tile([128, 64 * 64], F16, name="pw_t")    # pool-W out
            pw_v = pw_t[:].rearrange("p (s u) -> p s u", u=64)

            # consts to SBUF (after chunk-0 loads in queue order; tiny ones
            # go on the gpsimd SWDGE queue so they don't delay the scalar
            # queue's first front ops)
            w1p_sb = pp.tile([128, 384], F16, name="w1p_sb")
            nc.sync.dma_start(w1p_sb[:], w1p_d[:])
            w1s_sb = pp.tile([64, 384], F16, name="w1s_sb")
            nc.gpsimd.dma_start(w1s_sb[:], w1s_d[:])
            bnv = {}
            for n, d in bn_vecs.items():
                t = pp.tile([128, 1], F32, name=f"{n}_sb")
                nc.gpsimd.dma_start(t[:], d[:])
                bnv[n] = t

            # conv1 state (PSUM banks live across the whole front)
            a1_sb = pp.tile([128, 4096], F16, name="a1_sb")
            junk = pp.tile([128, 512], F32, name="junk")
            s1b = pp.tile([128, 8], F32, name="s1b")
            s2b = pp.tile([128, 8], F32, name="s2b")
            a1v = a1_sb[:].rearrange("p (r q) -> p r q", q=64)
            psB_pool = tc.tile_pool(name="psB", bufs=8, space="PSUM")
            psB = psB_pool.__enter__()
            ps_list = [psB.tile([128, 512], F32, tag="psB", name=f"c1ps{i}")
                       for i in range(8)]

            def conv1_chunk(ch):
                q0 = ch * 8
                for kx in range(3):
                    nc.tensor.matmul(ps_list[ch][:],
                                     w1s_sb[:, kx*128:(kx+1)*128],
                                     p1v[0:64, q0+2:q0+10, kx:kx+64],
                                     start=(kx == 0), stop=False)
                for kx in range(3):
                    nc.tensor.matmul(ps_list[ch][:],
                                     w1p_sb[:, kx*128:(kx+1)*128],
                                     p1v[:, q0:q0+8, kx:kx+64],
                                     start=False, stop=(kx == 2))
                # evacuate + BN1 stats (conv bias is a no-op before BN)
                nc.vector.tensor_scalar(a1v[:, q0:q0+8, :], ps_list[ch][:],
                                        1.0, 0.0, ALU.mult, ALU.add,
                                        accum_out=s1b[:, ch:ch+1])
                nc.scalar.activation(junk[:], ps_list[ch][:], AF.Square,
                                     accum_out=s2b[:, ch:ch+1])

            r30, r10, r02 = D3 / D2, D1 / D0, D0 / D2
            m0, m1 = (D1 + D2) / D2, (D0 + D3) / D2
            # DWT-W ts/tt chain ratios (stt runs at half DVE rate, so cheap
            # ops beat 3 stt): y'' = y/D1. The host pre-scales the O plane
            # by D0/D1, so u2 needs no trailing scale.
            w_s1, w_s2 = D3 / D1, D2 / D0
            mw0 = (D1 + D2) / D1
            mw1 = ((D0 + D3) / D1) * (D1 / D0)   # for the D0/D1-scaled O

            def dwt_h_piece(a, b, v1_p, v2_p):
                """y2''[s] for s_local in [a, b) (a >= 1), both part groups.
                v1 = (d3/d2) y'[2s-2] + y'[2s-1]; v2 = (d1/d0) y'[2s] + y'[2s+1]
                y2'' = (d0/d2) v2 + v1; scaled copies on ACT, adds on DVE"""
                n = b - a
                v1 = v1_p[:].rearrange("p (s t) -> p s t", t=128)[:, 0:n, :]
                v2 = v2_p[:].rearrange("p (s t) -> p s t", t=128)[:, 0:n, :]
                nc.vector.scalar_tensor_tensor(v1, y_v[:, 2*a-2:2*b-3:2, :],
                                               r30, y_v[:, 2*a-1:2*b-2:2, :],
                                               ALU.mult, ALU.add)
                nc.vector.scalar_tensor_tensor(v2, y_v[:, 2*a:2*b-1:2, :],
                                               r10, y_v[:, 2*a+1:2*b:2, :],
                                               ALU.mult, ALU.add)
                nc.vector.scalar_tensor_tensor(y2_v[:, a:b, :], v2, r02, v1,
                                               ALU.mult, ALU.add)

            conv1_done = set()

            def issue_conv1(chs):
                for ch in chs:
                    if ch not in conv1_done:
                        conv1_done.add(ch)
                        conv1_chunk(ch)

            with tc.tile_pool(name="twp", bufs=2) as twp, \
                 tc.tile_pool(name="thp", bufs=2) as thp:

                def wave(hc):
                    # DWT-H + pools + assembly for the s-range chunk hc
                    # completed; issued one chunk late so these DVE ops run
                    # while ACT produces the next chunk's scaled copies
                    a = max(1, 16 * hc)
                    b = 16 * hc + 16
                    v1_p = thp.tile([128, 16 * 128], F16, tag="v1")
                    v2_p = thp.tile([128, 16 * 128], F16, tag="v2")
                    dwt_h_piece(a, b, v1_p, v2_p)
                    if hc == 0:
                        # s_local=0, group 0 mirror: m0 y'[0] + m1 y'[1]
                        nc.vector.scalar_tensor_tensor(
                            y2_v[0:64, 0:1, :], y_v[0:64, 0:1, :], m0 / m1,
                            y_v[0:64, 1:2, :], ALU.mult, ALU.add)
                        nc.vector.tensor_scalar(y2_v[0:64, 0:1, :],
                                                y2_v[0:64, 0:1, :],
                                                m1, None, ALU.mult)
                        nc.vector.tensor_tensor(pw_v[0:64, 0:1, :],
                                                y2_v[0:64, 0:1, 0::2],
                                                y2_v[0:64, 0:1, 1::2], ALU.max)
                    # pool-W for those s rows (on the otherwise-idle Pool eng)
                    nc.vector.tensor_tensor(pw_v[:, a:b, :],
                                            y2_v[:, a:b, 0::2],
                                            y2_v[:, a:b, 1::2], ALU.max)
                    if hc == 3:
                        # s_local=0 group 1 seam: needs y' rows 126,127 of g0
                        seam = fp.tile([128, 2 * 128], F16, name="seam")
                        nc.sync.dma_start(seam[64:128, :],
                                          y_t[0:64, 126*128:128*128])
                        seam_v = seam[:].rearrange("p (h t) -> p h t", t=128)
                        sv1 = fp.tile([128, 128], F16, name="sv1")
                        sv2 = fp.tile([128, 128], F16, name="sv2")
                        nc.vector.scalar_tensor_tensor(
                            sv1[64:128, :].rearrange("p (a t) -> p a t", a=1),
                            seam_v[64:128, 0:1, :], r30,
                            seam_v[64:128, 1:2, :], ALU.mult, ALU.add)
                        nc.vector.scalar_tensor_tensor(
                            sv2[64:128, :].rearrange("p (a t) -> p a t", a=1),
                            y_v[64:128, 0:1, :], r10,
                            y_v[64:128, 1:2, :], ALU.mult, ALU.add)
                        nc.vector.scalar_tensor_tensor(
                            y2_v[64:128, 0:1, :],
                            sv2[64:128, :].rearrange("p (a t) -> p a t", a=1),
                            r02,
                            sv1[64:128, :].rearrange("p (a t) -> p a t", a=1),
                            ALU.mult, ALU.add)
                        nc.vector.tensor_tensor(pw_v[64:128, 0:1, :],
                                                y2_v[64:128, 0:1, 0::2],
                                                y2_v[64:128, 0:1, 1::2],
                                                ALU.max)
                        # deferred pooled row q=32 (block1 row 32) + its
                        # cross-partition copy into block0 row 33
                        nc.vector.tensor_tensor(p1v[64:128, 32:33, 1:65],
                                                pw_v[64:128, 0:1, :],
                                                pw_v[64:128, 1:2, :], ALU.max)
                        nc.sync.dma_start(p1v[0:64, 33:34, :],
                                          p1v[64:128, 32:33, :])
                    # pool-H for pooled rows this chunk completes:
                    # g0: q rows 8hc..8hc+7 -> p1 block0 rows 1+8hc..8+8hc
                    # g1: q rows 32+8hc..39+8hc -> p1 block1 rows 32+8hc..
                    # (g1 pooled row 32 needs the hc==3 seam -> deferred,
                    #  so at hc==0 only rows 33..39 are written here)
                    qa = 8 * hc
                    src0 = pw_v[0:64, 2*qa:2*qa+16, :]
                    nc.vector.tensor_tensor(p1v[0:64, 1+qa:9+qa, 1:65],
                                            src0[:, 0::2, :], src0[:, 1::2, :],
                                            ALU.max)
                    r1a = 33 if hc == 0 else 32 + qa    # block1 first row
                    src1 = pw_v[64:128, 2*(r1a-32):2*qa+16, :]
                    nc.vector.tensor_tensor(p1v[64:128, r1a:40+qa, 1:65],
                                            src1[:, 0::2, :], src1[:, 1::2, :],
                                            ALU.max)
                    # cross-partition copies for the packed layout
                    # block0 rows r1a+1..40+qa <- block1 rows r1a..39+qa
                    nc.sync.dma_start(p1v[0:64, r1a+1:41+qa, :],
                                      p1v[64:128, r1a:40+qa, :])
                    # block1 rows qa..qa+7 <- block0 rows 1+qa..8+qa
                    nc.scalar.dma_start(p1v[64:128, qa:qa+8, :],
                                        p1v[0:64, 1+qa:9+qa, :])
                    # conv1 chunks that just became ready
                    if stage > 2:
                        if hc == 1:
                            issue_conv1([0])
                        elif hc == 2:
                            issue_conv1([1, 5])

                tiles = {0: (xe0, xo0)}

                def load_chunk(h):
                    xe = xin.tile([128, 32 * 128], F16, tag="xe")
                    xo = xin.tile([128, 32 * 128], F16, tag="xo")
                    o0 = h * 4096
                    nc.sync.dma_start(xe[0:64], xg[0, 0, :, o0:o0+4096])
                    nc.scalar.dma_start(xe[64:128], xg[1, 0, :, o0:o0+4096])
                    nc.sync.dma_start(xo[0:64], xg[0, 1, :, o0:o0+4096])
                    nc.scalar.dma_start(xo[64:128], xg[1, 1, :, o0:o0+4096])
                    tiles[h] = (xe, xo)

                for hc in range(4):          # h-chunks of 32 rows
                    # prefetch the NEXT chunk now: dispatched ahead of this
                    # iteration's ACT ops, its data lands before the next
                    # iteration's scaled copies need it
                    if hc < 3:
                        load_chunk(hc + 1)
                    # previous chunk's DWT-H/pool wave first: its DVE ops are
                    # ready now and fill the DVE while ACT produces this
                    # chunk's scaled copies
                    if hc >= 1:
                        wave(hc - 1)
                    xe, xo = tiles.pop(hc)
                    ev = xe[:].rearrange("p (h w) -> p h w", w=128)
                    ov = xo[:].rearrange("p (h w) -> p h w", w=128)
                    yc = y_v[:, hc*32:(hc+1)*32, :]
                    # DWT-W 5-op chain (y'' = y/D1, O plane pre-scaled by
                    # D0/D1 on host): u1 = (D3/D1) E' + E''
                    # u2 = (D2/D0) Os' + Os'';  y'' = u1 + u2
                    # Scaled copies go to the idle ACT engine (except chunk
                    # 0, where ACT would delay the DVE start); chunk 0 runs
                    # in 16-row sub-chunks to shorten the lead-in.
                    for r0, rn in (((0, 16), (16, 16)) if hc == 0
                                   else ((0, 32),)):
                        evr = ev[:, r0:r0+rn, :]
                        ovr = ov[:, r0:r0+rn, :]
                        ycr = yc[:, r0:r0+rn, :]
                        u1 = twp.tile([128, 32 * 127], F16, tag="u1")
                        u2 = twp.tile([128, 32 * 127], F16, tag="u2")
                        u1v = u1[:].rearrange("p (h t) -> p h t",
                                              t=127)[:, 0:rn, :]
                        u2v = u2[:].rearrange("p (h t) -> p h t",
                                              t=127)[:, 0:rn, :]
                        if hc == 0:
                            nc.vector.tensor_scalar(u1v, evr[:, :, 0:127],
                                                    w_s1, None, ALU.mult)
                            nc.vector.tensor_scalar(u2v, ovr[:, :, 0:127],
                                                    w_s2, None, ALU.mult)
                        else:
                            nc.scalar.activation(u1v, evr[:, :, 0:127],
                                                 AF.Identity, scale=w_s1)
                            nc.scalar.activation(u2v, ovr[:, :, 0:127],
                                                 AF.Identity, scale=w_s2)
                        nc.vector.tensor_tensor(u1v, u1v, evr[:, :, 1:128],
                                                ALU.add)
                        nc.vector.tensor_tensor(u2v, u2v, ovr[:, :, 1:128],
                                                ALU.add)
                        nc.vector.tensor_tensor(ycr[:, :, 1:128], u1v, u2v,
                                                ALU.add)
                        # t=0 mirror: y''[0] = mw0 E[0] + mw1 Os[0]
                        nc.vector.scalar_tensor_tensor(
                            ycr[:, :, 0:1],
                            evr[:, :, 0:1], mw0 / mw1, ovr[:, :, 0:1],
                            ALU.mult, ALU.add)
                        nc.vector.tensor_scalar(ycr[:, :, 0:1],
                                                ycr[:, :, 0:1],
                                                mw1, None, ALU.mult)
                wave(3)
                # mid/back consts (queues free once front loads are done)
                w2t_sb = pp.tile([128, 1152], F16, name="w2t_sb")
                nc.sync.dma_start(w2t_sb[:], w2t_d[:])
                wdt_sb = pp.tile([128, 256], F16, name="wdt_sb")
                nc.scalar.dma_start(wdt_sb[:], wdt_d[:])
                db_sb = pp.tile([64, 1], F32, name="db_sb")
                nc.sync.dma_start(db_sb[:], db_d[:])
                bhw_sb = pp.tile([128, 254], F16, name="bhw_sb")
                nc.scalar.dma_start(bhw_sb[:], bhw_d[:])

            front_pool.__exit__(None, None, None)
            xin_pool.__exit__(None, None, None)
            if stage <= 2:
                psB_pool.__exit__(None, None, None)
                return

            # ---------------- conv1 rest (+BN1) ----------------
            mid_pool = tc.tile_pool(name="mid", bufs=1)
            mp = mid_pool.__enter__()
            in2_pad = mp.tile([128, 66 * 66], F16, name="in2_pad")
            nc.gpsimd.memset(in2_pad[:], 0.0)
            issue_conv1([2, 3, 4, 6, 7])
            psB_pool.__exit__(None, None, None)

            if stage <= 3:
                mid_pool.__exit__(None, None, None)
                return
            sc1, bi1 = _bn_coeffs(nc, pp, s1b, s2b, cc_bufs[1], rg, cnt,
                                  bnv["bn1g"], bnv["bn1b"], use_cc, tag=1)

            # BN1 + ReLU on DVE (faster than ACT, and DVE is idle here),
            # split in h-halves so conv2 chunks can start early
            p2v = in2_pad[:].rearrange("p (r v) -> p r v", v=66)

            def bn1_apply(hh):
                dst = p2v[:, 1+32*hh:33+32*hh, 1:65]
                nc.vector.tensor_scalar(dst, a1v[:, 32*hh:32*hh+32, :],
                                        sc1[:], bi1[:], ALU.mult, ALU.add)
                nc.vector.tensor_scalar_max(dst, dst, 0.0)

            if stage <= 4:
                bn1_apply(0)
                bn1_apply(1)
                mid_pool.__exit__(None, None, None)
                return
            # ---------------- conv2 (+BN2 stats) ----------------
            h2_sb = mp.tile([128, 4096], F16, name="h2_sb")
            h2v = h2_sb[:].rearrange("p (r q) -> p r q", q=64)
            s1c = pp.tile([128, 8], F32, name="s1c")
            s2c = pp.tile([128, 8], F32, name="s2c")
            with tc.tile_pool(name="psC", bufs=8, space="PSUM") as psC:
                ps_list2 = [psC.tile([128, 512], F32, tag="psC", name=f"c2ps{i}")
                            for i in range(8)]

                def conv2_chunk(ch):
                    p0 = ch * 8
                    for ti in range(9):
                        ky, kx = divmod(ti, 3)
                        rhs = p2v[:, p0+ky:p0+ky+8, kx:kx+64]
                        nc.tensor.matmul(ps_list2[ch][:],
                                         w2t_sb[:, ti*128:(ti+1)*128],
                                         rhs, start=(ti == 0), stop=(ti == 8))
                    nc.vector.tensor_scalar(h2v[:, p0:p0+8, :],
                                            ps_list2[ch][:],
                                            1.0, 0.0, ALU.mult, ALU.add,
                                            accum_out=s1c[:, ch:ch+1])
                    nc.scalar.activation(junk[:], ps_list2[ch][:], AF.Square,
                                         accum_out=s2c[:, ch:ch+1])

                bn1_apply(0)
                for ch in (0, 1, 2):
                    conv2_chunk(ch)
                bn1_apply(1)
                for ch in (3, 4, 5, 6, 7):
                    conv2_chunk(ch)

            sc2, bi2 = _bn_coeffs(nc, pp, s1c, s2c, cc_bufs[2], rg, cnt,
                                  bnv["bn2g"], bnv["bn2b"], use_cc, tag=2)

            def bn2_apply(r0, rn):
                dst = h2v[:, r0:r0+rn, :]
                nc.vector.tensor_scalar(dst, dst, sc2[:], bi2[:],
                                        ALU.mult, ALU.add)
                nc.vector.tensor_scalar_max(dst, dst, 0.0)

            if stage <= 5:
                bn2_apply(0, 64)
                mid_pool.__exit__(None, None, None)
                return
            # ---------------- deconv ----------------
            dth = pp.tile([128, 64 * 128], F16, name="dth")
            dth_v = dth[:].rearrange("p (o w) -> p o w", w=128)
            scr2_h = scr2[:].rearrange("o h w -> h o w")
            d_sb = mp.tile([64, 128 * 128], F16, name="d_sb")
            dv = d_sb[:].rearrange("p (h w) -> p h w", w=128)
            with tc.tile_pool(name="psD", bufs=8, space="PSUM") as psD:
                # scr2 writes ride the SWDGE queue; the descriptor-bound
                # transposed reads get both HWDGE queues, split 4-way so all
                # 16 SDMA engines stay fed
                for r0, rn in ((0, 32), (32, 32)):
                    bn2_apply(r0, rn)
                    for kl in range(4):
                        k, l = divmod(kl, 2)
                        for ch in range(rn // 8):
                            p0 = r0 + ch * 8
                            ps = psD.tile([64, 512], F32, tag="psD")
                            nc.tensor.matmul(ps[:], wdt_sb[:, kl*64:(kl+1)*64],
                                             h2v[:, p0:p0+8, :],
                                             start=True, stop=True)
                            dst = dv[:, 2*p0+k:2*p0+k+15:2, l::2]
                            if (kl * 4 + ch) % 2 == 0:
                                nc.vector.tensor_scalar(dst, ps[:], 1.0,
                                                        db_sb[:],
                                                        ALU.mult, ALU.add)
                            else:
                                nc.scalar.activation(dst, ps[:], AF.Identity,
                                                     bias=db_sb[:], scale=1.0)
                    h0, hn = 2 * r0, 2 * rn
                    nc.gpsimd.dma_start(scr2[:, h0:h0+hn, :],
                                        dv[:, h0:h0+hn, :])
                    for oq in range(4):
                        eng = nc.sync if oq % 2 == 0 else nc.scalar
                        eng.dma_start(dth_v[h0:h0+hn, oq*16:(oq+1)*16, :],
                                      scr2_h[h0:h0+hn, oq*16:(oq+1)*16, :])
            mid_pool.__exit__(None, None, None)
            if stage <= 6:
                return

            # ---------------- IDWT-H on PE, IDWT-W spread ----------------
            # g2 = REC2 * (IDWT-H of dth)  (REC2 folded into BHW)
            # out evens: o[2t] = g2[t] + (REC0/REC2) g2[t+1]   (one stt)
            # out odds:  o[2t+1] = (REC3/REC2) g2[t] + (REC1/REC2) g2[t+1]
            q0r, q1r, q3r = REC[0] / REC[2], REC[1] / REC[2], REC[3] / REC[2]
            with tc.tile_pool(name="psE", bufs=8, space="PSUM") as psE, \
                 tc.tile_pool(name="gpool", bufs=2) as gpool, \
                 tc.tile_pool(name="twpool", bufs=2) as twpool, \
                 tc.tile_pool(name="opool", bufs=3) as opool:
                for blk in range(2):
                    g_t = gpool.tile([127, 8192], F16, tag="g")
                    g_v = g_t[:].rearrange("p (o w) -> p o w", w=128)
                    for nch in range(16):
                        ps = psE.tile([127, 512], F32, tag="psE")
                        nc.tensor.matmul(ps[:], bhw_sb[:, blk*127:blk*127+127],
                                         dth[:, nch*512:(nch+1)*512],
                                         start=True, stop=True)
                        dst = g_t[:, nch*512:(nch+1)*512]
                        if nch % 2 == 0:
                            nc.vector.tensor_copy(dst, ps[:])
                        else:
                            nc.scalar.copy(dst, ps[:])
                    # blk1 split finer so the final store tail is short
                    pieces = ((0, 32), (32, 32)) if blk == 0 else \
                             ((0, 32), (32, 16), (48, 16))
                    for pi, (r0, rn) in enumerate(pieces):
                        gh = g_v[:, r0:r0+rn, :]
                        o_t = opool.tile([127, 32 * 254], F16, tag="o")
                        o_v = o_t[:].rearrange("p (o w) -> p o w",
                                               w=254)[:, 0:rn, :]
                        # ts/tt chains (stt runs at half DVE rate); the
                        # terminal odds op goes to ACT so nothing on the DVE
                        # critical path waits for the slower engine
                        # evens: o[2t] = q0r g2[t+1] + g2[t]
                        ae = twpool.tile([127, 32 * 127], F16, tag="ae")
                        aev = ae[:].rearrange("p (o w) -> p o w",
                                              w=127)[:, 0:rn, :]
                        nc.vector.tensor_scalar(aev, gh[:, :, 1:128], q0r,
                                                None, ALU.mult)
                        nc.vector.tensor_tensor(o_v[:, :, 0:253:2], aev,
                                                gh[:, :, 0:127], ALU.add)
                        # odds: o[2t+1] = q3r ((q1r/q3r) g2[t+1] + g2[t])
                        ao = twpool.tile([127, 32 * 127], F16, tag="ao")
                        aov = ao[:].rearrange("p (o w) -> p o w",
                                              w=127)[:, 0:rn, :]
                        nc.vector.tensor_scalar(aov, gh[:, :, 1:128], q1r/q3r,
                                                None, ALU.mult)
                        nc.vector.tensor_tensor(aov, aov, gh[:, :, 0:127],
                                                ALU.add)
                        nc.scalar.activation(o_v[:, :, 1:254:2], aov,
                                             AF.Identity, scale=q3r)
                        for i in range(8):
                            oi = blk * 8 + i
                            h0, sz = OUT_SPLITS[oi]
                            p0 = h0 - blk * 127
                            eng = nc.sync if (i + pi) % 2 == 0 else nc.scalar
                            eng.dma_start(out_ds[oi][:, r0:r0+rn, :],
                                          o_v[p0:p0+sz])

        _body()
    nc.compile()
    return nc


def _bn_coeffs(nc, pp, s1b, s2b, cc_pair, rg, cnt, g_sb, b_sb, use_cc, tag):
    """Reduce per-chunk sums, AllReduce across cores, return (scale, bias) [128,1]."""
    ALU = mybir.AluOpType
    sl = pp.tile([128, 2], F32, name=f"bn{tag}_sl")
    nc.vector.tensor_reduce(sl[:, 0:1], s1b[:], mybir.AxisListType.X, ALU.add)
    nc.vector.tensor_reduce(sl[:, 1:2], s2b[:], mybir.AxisListType.X, ALU.add)
    cc_in, cc_out = cc_pair
    sg = pp.tile([128, 2], F32, name=f"bn{tag}_sg")
    if use_cc:
        nc.sync.dma_start(cc_in[:], sl[:])
        nc.gpsimd.collective_compute(
            "AllReduce", ALU.add, replica_groups=rg,
            ins=[cc_in[:]], outs=[cc_out[:]])
        nc.sync.dma_start(sg[:], cc_out[:])
    else:
        nc.vector.tensor_copy(sg[:], sl[:])

    m = pp.tile([128, 1], F32, name=f"bn{tag}_m")
    vpe = pp.tile([128, 1], F32, name=f"bn{tag}_v")
    t0 = pp.tile([128, 1], F32, name=f"bn{tag}_t0")
    nc.vector.tensor_scalar(m[:], sg[:, 0:1], 1.0 / cnt, None, ALU.mult)
    nc.vector.tensor_tensor(t0[:], m[:], m[:], ALU.mult)          # m^2
    nc.vector.tensor_scalar(vpe[:], sg[:, 1:2], 1.0 / cnt, float(EPS), ALU.mult,
                            ALU.add)                              # E[x^2]+eps
    nc.vector.tensor_tensor(vpe[:], vpe[:], t0[:], ALU.subtract)  # var+eps
    # rsqrt with one Newton step (ACT Sqrt is low-precision)
    s0 = pp.tile([128, 1], F32, name=f"bn{tag}_s0")
    y0 = pp.tile([128, 1], F32, name=f"bn{tag}_y0")
    nc.scalar.activation(s0[:], vpe[:], mybir.ActivationFunctionType.Sqrt)
    nc.vector.reciprocal(y0[:], s0[:])
    t1 = pp.tile([128, 1], F32, name=f"bn{tag}_t1")
    nc.vector.tensor_tensor(t1[:], y0[:], y0[:], ALU.mult)
    nc.vector.tensor_tensor(t1[:], t1[:], vpe[:], ALU.mult)
    nc.vector.tensor_scalar(t1[:], t1[:], -0.5, 1.5, ALU.mult, ALU.add)
    nc.vector.tensor_tensor(y0[:], y0[:], t1[:], ALU.mult)        # refined rsqrt
    sc = pp.tile([128, 1], F32, name=f"bn{tag}_sc")
    bi = pp.tile([128, 1], F32, name=f"bn{tag}_bi")
    nc.vector.tensor_tensor(sc[:], y0[:], g_sb[:], ALU.mult)
    nc.vector.tensor_tensor(t0[:], m[:], sc[:], ALU.mult)
    nc.vector.tensor_tensor(bi[:], b_sb[:], t0[:], ALU.subtract)
    return sc, bi


# ---------------------------------------------------------------- entry point
_CACHE = {}


def kernel(x, conv1_w, conv1_b, bn1_g, bn1_b, conv2_w, conv2_b, bn2_g, bn2_b,
           deconv_w, deconv_b):
    world = N_CORES
    if "nc" not in _CACHE:
        _CACHE["nc"] = build_nc(world)
    nc = _CACHE["nc"]

    consts = pack_consts(np.asarray(conv1_w), np.asarray(conv2_w),
                         np.asarray(deconv_w), np.asarray(deconv_b),
                         np.asarray(bn1_g), np.asarray(bn1_b),
                         np.asarray(bn2_g), np.asarray(bn2_b))
    x = np.asarray(x)
    # host-side prep: f16 + even/odd w split -> [c, parity, h, 128];
    # the odd plane carries the D0/D1 factor of the DWT-W chain
    xs = np.stack([x[:, :, :, 0::2].astype(np.float16),
                   (x[:, :, :, 1::2] * (DEC[0] / DEC[1])).astype(np.float16)],
                  axis=2)
    in_maps = []
    for n in range(world):
        m = {"x": np.ascontiguousarray(xs[n])}
        m.update(consts)
        in_maps.append(m)

    res = run_bass_kernel_spmd(
        nc, in_maps, core_ids=list(range(world)),
        trace=bool(int(os.environ.get("WK_TRACE", "0"))))
    out = np.stack(
        [np.concatenate([r[f"out{i}"] for i in range(16)], axis=0).transpose(1, 0, 2)
         for r in res.results], axis=0).astype(np.float32)
    _CACHE["last_perf"] = res
    return out


# revision 68
# speedup vs baseline: 1.4748x; 1.0313x over previous
"""Trainium2 Bass kernel for nn_Center2D (DWT -> pool -> conv-BN-ReLU x2 -> deconv -> IDWT).

Self-contained: hardcodes shapes from the problem spec.
Sharding: pure data parallel, batch dim (8) across 8 cores; BN batch stats
synchronized with a tiny AllReduce (2x128 floats) per BN layer.

Layout strategy per core (one sample):
  io:    x is pre-split on host into even/odd w-columns and cast to f16
         (halves HBM read traffic and makes all front DVE reads
         contiguous); output is stored f16 and upcast on host.
  front: DWT-W as a 6-op ts/tt chain (stt runs at half DVE rate), scaled
         copies offloaded to the ACT engine, 1/D1*1/D2 fold absorbed into
         conv1 weights on host (positive, so max-pools commute); the
         DWT-H/pool wave trails one chunk so DVE never waits on ACT;
         conv1 matmuls issued per row-chunk overlap the front on the
         otherwise-idle PE.
  mid:   conv1 K-packed (ky=0,1 pairs -> 128-deep contraction, via a
         partition-shifted duplicate of the padded input), conv2 as 9
         K-packed matmuls, BN stats via accum_out during PSUM evacuation,
         tiny AllReduce per BN (warmed up by a dummy collective at t=0),
         BN+ReLU applied on DVE in halves so conv2/deconv start early.
  back:  deconv as 4 PE matmuls in h-halves, DRAM round-trip to put H on
         partitions (writes on the SWDGE queue, descriptor-bound
         transposed reads split 4-way over both HWDGE queues), PE matmul
         for IDWT-H with REC2 folded into the banded matrix, IDWT-W as
         ts/tt chains with the terminal odds op on ACT, f16 stores split
         across 16 ExternalOutput tensors.
"""

import os
import numpy as np

import concourse.bass as bass
import concourse.bacc as bacc
import concourse.tile as tile
from concourse import mybir
from concourse.bass_utils import run_bass_kernel_spmd

F32 = mybir.dt.float32
F16 = mybir.dt.float16
AF = mybir.ActivationFunctionType
ALU = mybir.AluOpType

REC = np.array([0.48296291314469025, 0.8365163037378079,
                0.22414386804185735, -0.12940952255092145], dtype=np.float64)
DEC = REC[::-1].copy()

N_CORES = int(os.environ.get("WK_CORES", "8"))
EPS = 1e-5

D0, D1, D2, D3 = (float(DEC[0]), float(DEC[1]), float(DEC[2]), float(DEC[3]))
# DWT-W chain carries 1/D1, DWT-H chain 1/D2 -> fold D1*D2 (positive, so
# the max-pools commute) into conv1 weights
FOLD = D1 * D2


# ---------------------------------------------------------------- host consts
def build_BH():
    """IDWT along one axis as a dense [128, 254] matrix, pre-scaled by REC2
    so the even-column IDWT-W tap needs no temporary."""
    B = np.zeros((128, 254), dtype=np.float64)
    for t in range(127):
        B[t,   2*t] += REC[2]
        B[t+1, 2*t] += REC[0]
        B[t,   2*t+1] += REC[3]
        B[t+1, 2*t+1] += REC[1]
    return (B * REC[2]).astype(np.float32)


def pack_consts(conv1_w, conv2_w, deconv_w, deconv_b,
                bn1_g, bn1_b, bn2_g, bn2_b):
    bhw = build_BH().astype(np.float16)          # [128, 254]

    w1 = conv1_w.astype(np.float64) * FOLD       # fold DWT chain scale
    # packed ky=0/1 pairs: rows 0:64 = ci(ky=0), 64:128 = ci(ky=1)
    w1p = np.zeros((128, 3 * 128), np.float16)
    w1s = np.zeros((64, 3 * 128), np.float16)    # ky=2
    for kx in range(3):
        w1p[0:64, kx*128:(kx+1)*128] = w1[:, :, 0, kx].T
        w1p[64:128, kx*128:(kx+1)*128] = w1[:, :, 1, kx].T
        w1s[:, kx*128:(kx+1)*128] = w1[:, :, 2, kx].T

    w2t = np.zeros((128, 9 * 128), np.float16)
    for ky in range(3):
        for kx in range(3):
            w2t[:, (ky*3+kx)*128:(ky*3+kx+1)*128] = conv2_w[:, :, ky, kx].T

    wdt = np.zeros((128, 4 * 64), np.float16)    # [ci, (k,l,o)]
    for k in range(2):
        for l in range(2):
            wdt[:, (k*2+l)*64:(k*2+l+1)*64] = deconv_w[:, :, k, l]

    return {
        "BHW": bhw,
        "w1p": w1p,
        "w1s": w1s,
        "w2t": w2t,
        "wdt": wdt,
        "db": deconv_b.reshape(64, 1).astype(np.float32),
        "bn1g": bn1_g.reshape(128, 1).astype(np.float32),
        "bn1b": bn1_b.reshape(128, 1).astype(np.float32),
        "bn2g": bn2_g.reshape(128, 1).astype(np.float32),
        "bn2b": bn2_b.reshape(128, 1).astype(np.float32),
    }


# ---------------------------------------------------------------- bass kernel
def build_nc(world=N_CORES, stage=None):
    if stage is None:
        stage = int(os.environ.get("WK_STAGE", "99"))
    nc = bacc.Bacc("TRN2", target_bir_lowering=False)
    use_cc = world > 1

    # x pre-split on host: [c, parity, h, 128] f16 (parity 0 = even w cols)
    x = nc.dram_tensor("x", (64, 2, 256, 128), F16, kind="ExternalInput")
    bhw_d = nc.dram_tensor("BHW", (128, 254), F16, kind="ExternalInput")
    w1p_d = nc.dram_tensor("w1p", (128, 384), F16, kind="ExternalInput")
    w1s_d = nc.dram_tensor("w1s", (64, 384), F16, kind="ExternalInput")
    w2t_d = nc.dram_tensor("w2t", (128, 1152), F16, kind="ExternalInput")
    wdt_d = nc.dram_tensor("wdt", (128, 256), F16, kind="ExternalInput")
    db_d = nc.dram_tensor("db", (64, 1), F32, kind="ExternalInput")
    bn_vecs = {n: nc.dram_tensor(n, (128, 1), F32, kind="ExternalInput")
               for n in ("bn1g", "bn1b", "bn2g", "bn2b")}
    # 16 separate f16 output tensors (h-slices); host upcasts to f32
    OUT_SPLITS = []
    h0 = 0
    for i in range(16):
        sz = 16 if (i % 8) != 7 else 15
        OUT_SPLITS.append((h0, sz))
        h0 += sz
    out_ds = [nc.dram_tensor(f"out{i}", (sz, 64, 254), F16, kind="ExternalOutput")
              for i, (_, sz) in enumerate(OUT_SPLITS)]

    scr2 = nc.dram_tensor("scr2", (64, 128, 128), F16, kind="Internal")
    cc_bufs = []
    for i in (0, 1, 2):
        cc_bufs.append((
            nc.dram_tensor(f"bn{i}_in", (128, 2), F32, kind="Internal"),
            nc.dram_tensor(f"bn{i}_out", (128, 2), F32, kind="Internal",
                           addr_space="Shared"),
        ))
    rg = [list(range(world))]
    cnt = float(world * 64 * 64)

    with tile.TileContext(nc) as tc, \
         tc.tile_pool(name="persist", bufs=1) as pp:
        def _body():
            # warmup collective: absorbs the ~11us first-call ncfw setup
            if use_cc:
                nc.gpsimd.collective_compute(
                    "AllReduce", ALU.add, replica_groups=rg,
                    ins=[cc_bufs[0][0][:]], outs=[cc_bufs[0][1][:]])

            # x viewed as [(g c) -> 128 partitions, h_local, w2]; partition
            # group g=0 holds h 0:128, g=1 holds h 128:256 of channel c
            xg = x[:].rearrange("c p (g r) w -> g p c (r w)", g=2)  # [2,2,64,128*128]

            # ---------- chunk 0 loads first (heads the dependency graph)
            xin_pool = tc.tile_pool(name="xin", bufs=2)
            xin = xin_pool.__enter__()
            xe0 = xin.tile([128, 32 * 128], F16, tag="xe")
            xo0 = xin.tile([128, 32 * 128], F16, tag="xo")
            # chunk 0 in 16-row pieces so the first DVE op starts sooner
            for c0 in (0, 2048):
                nc.sync.dma_start(xe0[0:64, c0:c0+2048],
                                  xg[0, 0, :, c0:c0+2048])
                nc.scalar.dma_start(xe0[64:128, c0:c0+2048],
                                    xg[1, 0, :, c0:c0+2048])
                nc.sync.dma_start(xo0[0:64, c0:c0+2048],
                                  xg[0, 1, :, c0:c0+2048])
                nc.scalar.dma_start(xo0[64:128, c0:c0+2048],
                                    xg[1, 1, :, c0:c0+2048])

            # conv1 input, with partition-shifted duplicate for K-packing:
            # parts 0:64 row r = padded row r; parts 64:128 row r = padded r+1
            in1b = pp.tile([128, 66 * 66], F16, name="in1b")
            nc.gpsimd.memset(in1b[:], 0.0)
            p1v = in1b[:].rearrange("p (r v) -> p r v", v=66)

            # ---------------- front: DWT + pool on DVE/GpSimd ----------
            front_pool = tc.tile_pool(name="front", bufs=1)
            fp = front_pool.__enter__()

            y_t = fp.tile([128, 128 * 128], F16, name="y_t")    # DWT-W out (y')
            y_v = y_t[:].rearrange("p (h t) -> p h t", t=128)
            y2_t = fp.tile([128, 64 * 128], F16, name="y2_t")   # DWT-H out (y2'')
            y2_v = y2_t[:].rearrange("p (s t) -> p s t", t=128)
            pw_t = fp.tile([128, 64 * 64], F16, name="pw_t")    # pool-W out
            pw_v = pw_t[:].rearrange("p (s u) -> p s u", u=64)

            # consts to SBUF (after chunk-0 loads in queue order; tiny ones
            # go on the gpsimd SWDGE queue so they don't delay the scalar
            # queue's first front ops)
            w1p_sb = pp.tile([128, 384], F16, name="w1p_sb")
            nc.sync.dma_start(w1p_sb[:], w1p_d[:])
            w1s_sb = pp.tile([64, 384], F16, name="w1s_sb")
            nc.gpsimd.dma_start(w1s_sb[:], w1s_d[:])
            bnv = {}
            for n, d in bn_vecs.items():
                t = pp.tile([128, 1], F32, name=f"{n}_sb")
                nc.gpsimd.dma_start(t[:], d[:])
                bnv[n] = t

            # conv1 state (PSUM banks live across the whole front)
            a1_sb = pp.tile([128, 4096], F16, name="a1_sb")
            junk = pp.tile([128, 512], F32, name="junk")
            s1b = pp.tile([128, 8], F32, name="s1b")
            s2b = pp.tile([128, 8], F32, name="s2b")
            a1v = a1_sb[:].rearrange("p (r q) -> p r q", q=64)
            psB_pool = tc.tile_pool(name="psB", bufs=8, space="PSUM")
            psB = psB_pool.__enter__()
            ps_list = [psB.tile([128, 512], F32, tag="psB", name=f"c1ps{i}")
                       for i in range(8)]

            def conv1_chunk(ch):
                q0 = ch * 8
                for kx in range(3):
                    nc.tensor.matmul(ps_list[ch][:],
                                     w1s_sb[:, kx*128:(kx+1)*128],
                                     p1v[0:64, q0+2:q0+10, kx:kx+64],
                                     start=(kx == 0), stop=False)
                for kx in range(3):
                    nc.tensor.matmul(ps_list[ch][:],
                                     w1p_sb[:, kx*128:(kx+1)*128],
                                     p1v[:, q0:q0+8, kx:kx+64],
                                     start=False, stop=(kx == 2))
                # evacuate + BN1 stats (conv bias is a no-op before BN)
                nc.vector.tensor_scalar(a1v[:, q0:q0+8, :], ps_list[ch][:],
                                        1.0, 0.0, ALU.mult, ALU.add,
                                        accum_out=s1b[:, ch:ch+1])
                nc.scalar.activation(junk[:], ps_list[ch][:], AF.Square,
                                     accum_out=s2b[:, ch:ch+1])

            r30, r10, r02 = D3 / D2, D1 / D0, D0 / D2
            m0, m1 = (D1 + D2) / D2, (D0 + D3) / D2
            # DWT-W ts/tt chain ratios (stt runs at half DVE rate, so cheap
            # ops beat 3 stt): y'' = y/D1. The host pre-scales the O plane
            # by D0/D1, so u2 needs no trailing scale.
            w_s1, w_s2 = D3 / D1, D2 / D0
            mw0 = (D1 + D2) / D1
            mw1 = ((D0 + D3) / D1) * (D1 / D0)   # for the D0/D1-scaled O

            def dwt_h_piece(a, b, v1_p, v2_p):
                """y2''[s] for s_local in [a, b) (a >= 1), both part groups.
                v1 = (d3/d2) y'[2s-2] + y'[2s-1]; v2 = (d1/d0) y'[2s] + y'[2s+1]
                y2'' = (d0/d2) v2 + v1; scaled copies on ACT, adds on DVE"""
                n = b - a
                v1 = v1_p[:].rearrange("p (s t) -> p s t", t=128)[:, 0:n, :]
                v2 = v2_p[:].rearrange("p (s t) -> p s t", t=128)[:, 0:n, :]
                nc.vector.scalar_tensor_tensor(v1, y_v[:, 2*a-2:2*b-3:2, :],
                                               r30, y_v[:, 2*a-1:2*b-2:2, :],
                                               ALU.mult, ALU.add)
                nc.vector.scalar_tensor_tensor(v2, y_v[:, 2*a:2*b-1:2, :],
                                               r10, y_v[:, 2*a+1:2*b:2, :],
                                               ALU.mult, ALU.add)
                nc.vector.scalar_tensor_tensor(y2_v[:, a:b, :], v2, r02, v1,
                                               ALU.mult, ALU.add)

            conv1_done = set()

            def issue_conv1(chs):
                for ch in chs:
                    if ch not in conv1_done:
                        conv1_done.add(ch)
                        conv1_chunk(ch)

            with tc.tile_pool(name="twp", bufs=2) as twp, \
                 tc.tile_pool(name="thp", bufs=2) as thp:

                def wave(hc):
                    # DWT-H + pools + assembly for the s-range chunk hc
                    # completed; issued one chunk late so these DVE ops run
                    # while ACT produces the next chunk's scaled copies
                    a = max(1, 16 * hc)
                    b = 16 * hc + 16
                    v1_p = thp.tile([128, 16 * 128], F16, tag="v1")
                    v2_p = thp.tile([128, 16 * 128], F16, tag="v2")
                    dwt_h_piece(a, b, v1_p, v2_p)
                    if hc == 0:
                        # s_local=0, group 0 mirror: m0 y'[0] + m1 y'[1]
                        nc.vector.scalar_tensor_tensor(
                            y2_v[0:64, 0:1, :], y_v[0:64, 0:1, :], m0 / m1,
                            y_v[0:64, 1:2, :], ALU.mult, ALU.add)
                        nc.vector.tensor_scalar(y2_v[0:64, 0:1, :],
                                                y2_v[0:64, 0:1, :],
                                                m1, None, ALU.mult)
                        nc.vector.tensor_tensor(pw_v[0:64, 0:1, :],
                                                y2_v[0:64, 0:1, 0::2],
                                                y2_v[0:64, 0:1, 1::2], ALU.max)
                    # pool-W for those s rows (on the otherwise-idle Pool eng)
                    nc.vector.tensor_tensor(pw_v[:, a:b, :],
                                            y2_v[:, a:b, 0::2],
                                            y2_v[:, a:b, 1::2], ALU.max)
                    if hc == 3:
                        # s_local=0 group 1 seam: needs y' rows 126,127 of g0
                        seam = fp.tile([128, 2 * 128], F16, name="seam")
                        nc.sync.dma_start(seam[64:128, :],
                                          y_t[0:64, 126*128:128*128])
                        seam_v = seam[:].rearrange("p (h t) -> p h t", t=128)
                        sv1 = fp.tile([128, 128], F16, name="sv1")
                        sv2 = fp.tile([128, 128], F16, name="sv2")
                        nc.vector.scalar_tensor_tensor(
                            sv1[64:128, :].rearrange("p (a t) -> p a t", a=1),
                            seam_v[64:128, 0:1, :], r30,
                            seam_v[64:128, 1:2, :], ALU.mult, ALU.add)
                        nc.vector.scalar_tensor_tensor(
                            sv2[64:128, :].rearrange("p (a t) -> p a t", a=1),
                            y_v[64:128, 0:1, :], r10,
                            y_v[64:128, 1:2, :], ALU.mult, ALU.add)
                        nc.vector.scalar_tensor_tensor(
                            y2_v[64:128, 0:1, :],
                            sv2[64:128, :].rearrange("p (a t) -> p a t", a=1),
                            r02,
                            sv1[64:128, :].rearrange("p (a t) -> p a t", a=1),
                            ALU.mult, ALU.add)
                        nc.vector.tensor_tensor(pw_v[64:128, 0:1, :],
                                                y2_v[64:128, 0:1, 0::2],
                                                y2_v[64:128, 0:1, 1::2],
                                                ALU.max)
                        # deferred pooled row q=32 (block1 row 32) + its
                        # cross-partition copy into block0 row 33
                        nc.vector.tensor_tensor(p1v[64:128, 32:33, 1:65],
                                                pw_v[64:128, 0:1, :],
                                                pw_v[64:128, 1:2, :], ALU.max)
                        nc.sync.dma_start(p1v[0:64, 33:34, :],
                                          p1v[64:128, 32:33, :])
                    # pool-H for pooled rows this chunk completes:
                    # g0: q rows 8hc..8hc+7 -> p1 block0 rows 1+8hc..8+8hc
                    # g1: q rows 32+8hc..39+8hc -> p1 block1 rows 32+8hc..
                    # (g1 pooled row 32 needs the hc==3 seam -> deferred,
                    #  so at hc==0 only rows 33..39 are written here)
                    qa = 8 * hc
                    src0 = pw_v[0:64, 2*qa:2*qa+16, :]
                    nc.vector.tensor_tensor(p1v[0:64, 1+qa:9+qa, 1:65],
                                            src0[:, 0::2, :], src0[:, 1::2, :],
                                            ALU.max)
                    r1a = 33 if hc == 0 else 32 + qa    # block1 first row
                    src1 = pw_v[64:128, 2*(r1a-32):2*qa+16, :]
                    nc.vector.tensor_tensor(p1v[64:128, r1a:40+qa, 1:65],
                                            src1[:, 0::2, :], src1[:, 1::2, :],
                                            ALU.max)
                    # cross-partition copies for the packed layout
                    # block0 rows r1a+1..40+qa <- block1 rows r1a..39+qa
                    nc.sync.dma_start(p1v[0:64, r1a+1:41+qa, :],
                                      p1v[64:128, r1a:40+qa, :])
                    # block1 rows qa..qa+7 <- block0 rows 1+qa..8+qa
                    nc.scalar.dma_start(p1v[64:128, qa:qa+8, :],
                                        p1v[0:64, 1+qa:9+qa, :])
                    # conv1 chunks that just became ready
                    if stage > 2:
                        if hc == 1:
                            issue_conv1([0])
                        elif hc == 2:
                            issue_conv1([1, 5])

                for hc in range(4):          # h-chunks of 32 rows
                    if hc == 0:
                        xe, xo = xe0, xo0
                    else:
                        xe = xin.tile([128, 32 * 128], F16, tag="xe")
                        xo = xin.tile([128, 32 * 128], F16, tag="xo")
                        o0 = hc * 4096
                        nc.sync.dma_start(xe[0:64], xg[0, 0, :, o0:o0+4096])
                        nc.scalar.dma_start(xe[64:128], xg[1, 0, :, o0:o0+4096])
                        nc.sync.dma_start(xo[0:64], xg[0, 1, :, o0:o0+4096])
                        nc.scalar.dma_start(xo[64:128], xg[1, 1, :, o0:o0+4096])
                    # previous chunk's DWT-H/pool wave first: its DVE ops are
                    # ready now and fill the DVE while ACT produces this
                    # chunk's scaled copies
                    if hc >= 1:
                        wave(hc - 1)
                    ev = xe[:].rearrange("p (h w) -> p h w", w=128)
                    ov = xo[:].rearrange("p (h w) -> p h w", w=128)
                    yc = y_v[:, hc*32:(hc+1)*32, :]
                    # DWT-W 5-op chain (y'' = y/D1, O plane pre-scaled by
                    # D0/D1 on host): u1 = (D3/D1) E' + E''
                    # u2 = (D2/D0) Os' + Os'';  y'' = u1 + u2
                    # Scaled copies go to the idle ACT engine (except chunk
                    # 0, where ACT would delay the DVE start); chunk 0 runs
                    # in 16-row sub-chunks to shorten the lead-in.
                    for r0, rn in (((0, 16), (16, 16)) if hc == 0
                                   else ((0, 32),)):
                        evr = ev[:, r0:r0+rn, :]
                        ovr = ov[:, r0:r0+rn, :]
                        ycr = yc[:, r0:r0+rn, :]
                        u1 = twp.tile([128, 32 * 127], F16, tag="u1")
                        u2 = twp.tile([128, 32 * 127], F16, tag="u2")
                        u1v = u1[:].rearrange("p (h t) -> p h t",
                                              t=127)[:, 0:rn, :]
                        u2v = u2[:].rearrange("p (h t) -> p h t",
                                              t=127)[:, 0:rn, :]
                        if hc == 0:
                            nc.vector.tensor_scalar(u1v, evr[:, :, 0:127],
                                                    w_s1, None, ALU.mult)
                            nc.vector.tensor_scalar(u2v, ovr[:, :, 0:127],
                                                    w_s2, None, ALU.mult)
                        else:
                            nc.scalar.activation(u1v, evr[:, :, 0:127],
                                                 AF.Identity, scale=w_s1)
                            nc.scalar.activation(u2v, ovr[:, :, 0:127],
                                                 AF.Identity, scale=w_s2)
                        nc.vector.tensor_tensor(u1v, u1v, evr[:, :, 1:128],
                                                ALU.add)
                        nc.vector.tensor_tensor(u2v, u2v, ovr[:, :, 1:128],
                                                ALU.add)
                        nc.vector.tensor_tensor(ycr[:, :, 1:128], u1v, u2v,
                                                ALU.add)
                        # t=0 mirror: y''[0] = mw0 E[0] + mw1 Os[0]
                        nc.vector.scalar_tensor_tensor(
                            ycr[:, :, 0:1],
                            evr[:, :, 0:1], mw0 / mw1, ovr[:, :, 0:1],
                            ALU.mult, ALU.add)
                        nc.vector.tensor_scalar(ycr[:, :, 0:1],
                                                ycr[:, :, 0:1],
                                                mw1, None, ALU.mult)
                wave(3)
                # mid/back consts (queues free once front loads are done)
                w2t_sb = pp.tile([128, 1152], F16, name="w2t_sb")
                nc.sync.dma_start(w2t_sb[:], w2t_d[:])
                wdt_sb = pp.tile([128, 256], F16, name="wdt_sb")
                nc.scalar.dma_start(wdt_sb[:], wdt_d[:])
                db_sb = pp.tile([64, 1], F32, name="db_sb")
                nc.sync.dma_start(db_sb[:], db_d[:])
                bhw_sb = pp.tile([128, 254], F16, name="bhw_sb")
                nc.scalar.dma_start(bhw_sb[:], bhw_d[:])

            front_pool.__exit__(None, None, None)
            xin_pool.__exit__(None, None, None)
            if stage <= 2:
                psB_pool.__exit__(None, None, None)
                return

            # ---------------- conv1 rest (+BN1) ----------------
            mid_pool = tc.tile_pool(name="mid", bufs=1)
            mp = mid_pool.__enter__()
            in2_pad = mp.tile([128, 66 * 66], F16, name="in2_pad")
            nc.gpsimd.memset(in2_pad[:], 0.0)
            issue_conv1([2, 3, 4, 6, 7])
            psB_pool.__exit__(None, None, None)

            if stage <= 3:
                mid_pool.__exit__(None, None, None)
                return
            sc1, bi1 = _bn_coeffs(nc, pp, s1b, s2b, cc_bufs[1], rg, cnt,
                                  bnv["bn1g"], bnv["bn1b"], use_cc, tag=1)

            # BN1 + ReLU on DVE (faster than ACT, and DVE is idle here),
            # split in h-halves so conv2 chunks can start early
            p2v = in2_pad[:].rearrange("p (r v) -> p r v", v=66)

            def bn1_apply(hh):
                dst = p2v[:, 1+32*hh:33+32*hh, 1:65]
                nc.vector.tensor_scalar(dst, a1v[:, 32*hh:32*hh+32, :],
                                        sc1[:], bi1[:], ALU.mult, ALU.add)
                nc.vector.tensor_scalar_max(dst, dst, 0.0)

            if stage <= 4:
                bn1_apply(0)
                bn1_apply(1)
                mid_pool.__exit__(None, None, None)
                return
            # ---------------- conv2 (+BN2 stats) ----------------
            h2_sb = mp.tile([128, 4096], F16, name="h2_sb")
            h2v = h2_sb[:].rearrange("p (r q) -> p r q", q=64)
            s1c = pp.tile([128, 8], F32, name="s1c")
            s2c = pp.tile([128, 8], F32, name="s2c")
            with tc.tile_pool(name="psC", bufs=8, space="PSUM") as psC:
                ps_list2 = [psC.tile([128, 512], F32, tag="psC", name=f"c2ps{i}")
                            for i in range(8)]

                def conv2_chunk(ch):
                    p0 = ch * 8
                    for ti in range(9):
                        ky, kx = divmod(ti, 3)
                        rhs = p2v[:, p0+ky:p0+ky+8, kx:kx+64]
                        nc.tensor.matmul(ps_list2[ch][:],
                                         w2t_sb[:, ti*128:(ti+1)*128],
                                         rhs, start=(ti == 0), stop=(ti == 8))
                    nc.vector.tensor_scalar(h2v[:, p0:p0+8, :],
                                            ps_list2[ch][:],
                                            1.0, 0.0, ALU.mult, ALU.add,
                                            accum_out=s1c[:, ch:ch+1])
                    nc.scalar.activation(junk[:], ps_list2[ch][:], AF.Square,
                                         accum_out=s2c[:, ch:ch+1])

                bn1_apply(0)
                for ch in (0, 1, 2):
                    conv2_chunk(ch)
                bn1_apply(1)
                for ch in (3, 4, 5, 6, 7):
                    conv2_chunk(ch)

            sc2, bi2 = _bn_coeffs(nc, pp, s1c, s2c, cc_bufs[2], rg, cnt,
                                  bnv["bn2g"], bnv["bn2b"], use_cc, tag=2)

            def bn2_apply(r0, rn):
                dst = h2v[:, r0:r0+rn, :]
                nc.vector.tensor_scalar(dst, dst, sc2[:], bi2[:],
                                        ALU.mult, ALU.add)
                nc.vector.tensor_scalar_max(dst, dst, 0.0)

            if stage <= 5:
                bn2_apply(0, 64)
                mid_pool.__exit__(None, None, None)
                return
            # ---------------- deconv ----------------
            dth = pp.tile([128, 64 * 128], F16, name="dth")
            dth_v = dth[:].rearrange("p (o w) -> p o w", w=128)
            scr2_h = scr2[:].rearrange("o h w -> h o w")
            d_sb = mp.tile([64, 128 * 128], F16, name="d_sb")
            dv = d_sb[:].rearrange("p (h w) -> p h w", w=128)
            with tc.tile_pool(name="psD", bufs=8, space="PSUM") as psD:
                # scr2 writes ride the SWDGE queue; the descriptor-bound
                # transposed reads get both HWDGE queues, split 4-way so all
                # 16 SDMA engines stay fed
                for r0, rn in ((0, 32), (32, 32)):
                    bn2_apply(r0, rn)
                    for kl in range(4):
                        k, l = divmod(kl, 2)
                        for ch in range(rn // 8):
                            p0 = r0 + ch * 8
                            ps = psD.tile([64, 512], F32, tag="psD")
                            nc.tensor.matmul(ps[:], wdt_sb[:, kl*64:(kl+1)*64],
                                             h2v[:, p0:p0+8, :],
                                             start=True, stop=True)
                            dst = dv[:, 2*p0+k:2*p0+k+15:2, l::2]
                            if (kl * 4 + ch) % 2 == 0:
                                nc.vector.tensor_scalar(dst, ps[:], 1.0,
                                                        db_sb[:],
                                                        ALU.mult, ALU.add)
                            else:
                                nc.scalar.activation(dst, ps[:], AF.Identity,
                                                     bias=db_sb[:], scale=1.0)
                    h0, hn = 2 * r0, 2 * rn
                    nc.gpsimd.dma_start(scr2[:, h0:h0+hn, :],
                                        dv[:, h0:h0+hn, :])
                    for oq in range(4):
                        eng = nc.sync if oq % 2 == 0 else nc.scalar
                        eng.dma_start(dth_v[h0:h0+hn, oq*16:(oq+1)*16, :],
                                      scr2_h[h0:h0+hn, oq*16:(oq+1)*16, :])
            mid_pool.__exit__(None, None, None)
            if stage <= 6:
                return

            # ---------------- IDWT-H on PE, IDWT-W spread ----------------
            # g2 = REC2 * (IDWT-H of dth)  (REC2 folded into BHW)
            # out evens: o[2t] = g2[t] + (REC0/REC2) g2[t+1]   (one stt)
            # out odds:  o[2t+1] = (REC3/REC2) g2[t] + (REC1/REC2) g2[t+1]
            q0r, q1r, q3r = REC[0] / REC[2], REC[1] / REC[2], REC[3] / REC[2]
            with tc.tile_pool(name="psE", bufs=8, space="PSUM") as psE, \
                 tc.tile_pool(name="gpool", bufs=2) as gpool, \
                 tc.tile_pool(name="twpool", bufs=2) as twpool, \
                 tc.tile_pool(name="opool", bufs=3) as opool:
                for blk in range(2):
                    g_t = gpool.tile([127, 8192], F16, tag="g")
                    g_v = g_t[:].rearrange("p (o w) -> p o w", w=128)
                    for nch in range(16):
                        ps = psE.tile([127, 512], F32, tag="psE")
                        nc.tensor.matmul(ps[:], bhw_sb[:, blk*127:blk*127+127],
                                         dth[:, nch*512:(nch+1)*512],
                                         start=True, stop=True)
                        dst = g_t[:, nch*512:(nch+1)*512]
                        if nch % 2 == 0:
                            nc.vector.tensor_copy(dst, ps[:])
                        else:
                            nc.scalar.copy(dst, ps[:])
                    # blk1 split finer so the final store tail is short
                    pieces = ((0, 32), (32, 32)) if blk == 0 else \
                             ((0, 32), (32, 16), (48, 16))
                    for pi, (r0, rn) in enumerate(pieces):
                        gh = g_v[:, r0:r0+rn, :]
                        o_t = opool.tile([127, 32 * 254], F16, tag="o")
                        o_v = o_t[:].rearrange("p (o w) -> p o w",
                                               w=254)[:, 0:rn, :]
                        # ts/tt chains (stt runs at half DVE rate); the
                        # terminal odds op goes to ACT so nothing on the DVE
                        # critical path waits for the slower engine
                        # evens: o[2t] = q0r g2[t+1] + g2[t]
                        ae = twpool.tile([127, 32 * 127], F16, tag="ae")
                        aev = ae[:].rearrange("p (o w) -> p o w",
                                              w=127)[:, 0:rn, :]
                        nc.vector.tensor_scalar(aev, gh[:, :, 1:128], q0r,
                                                None, ALU.mult)
                        nc.vector.tensor_tensor(o_v[:, :, 0:253:2], aev,
                                                gh[:, :, 0:127], ALU.add)
                        # odds: o[2t+1] = q3r ((q1r/q3r) g2[t+1] + g2[t])
                        ao = twpool.tile([127, 32 * 127], F16, tag="ao")
                        aov = ao[:].rearrange("p (o w) -> p o w",
                                              w=127)[:, 0:rn, :]
                        nc.vector.tensor_scalar(aov, gh[:, :, 1:128], q1r/q3r,
                                                None, ALU.mult)
                        nc.vector.tensor_tensor(aov, aov, gh[:, :, 0:127],
                                                ALU.add)
                        nc.scalar.activation(o_v[:, :, 1:254:2], aov,
                                             AF.Identity, scale=q3r)
                        for i in range(8):
                            oi = blk * 8 + i
                            h0, sz = OUT_SPLITS[oi]
                            p0 = h0 - blk * 127
                            eng = nc.sync if (i + pi) % 2 == 0 else nc.scalar
                            eng.dma_start(out_ds[oi][:, r0:r0+rn, :],
                                          o_v[p0:p0+sz])

        _body()
    nc.compile()
    return nc


def _bn_coeffs(nc, pp, s1b, s2b, cc_pair, rg, cnt, g_sb, b_sb, use_cc, tag):
    """Reduce per-chunk sums, AllReduce across cores, return (scale, bias) [128,1]."""
    ALU = mybir.AluOpType
    sl = pp.tile([128, 2], F32, name=f"bn{tag}_sl")
    nc.vector.tensor_reduce(sl[:, 0:1], s1b[:], mybir.AxisListType.X, ALU.add)
    nc.vector.tensor_reduce(sl[:, 1:2], s2b[:], mybir.AxisListType.X, ALU.add)
    cc_in, cc_out = cc_pair
    sg = pp.tile([128, 2], F32, name=f"bn{tag}_sg")
    if use_cc:
        nc.sync.dma_start(cc_in[:], sl[:])
        nc.gpsimd.collective_compute(
            "AllReduce", ALU.add, replica_groups=rg,
            ins=[cc_in[:]], outs=[cc_out[:]])
        nc.sync.dma_start(sg[:], cc_out[:])
    else:
        nc.vector.tensor_copy(sg[:], sl[:])

    m = pp.tile([128, 1], F32, name=f"bn{tag}_m")
    vpe = pp.tile([128, 1], F32, name=f"bn{tag}_v")
    t0 = pp.tile([128, 1], F32, name=f"bn{tag}_t0")
    nc.vector.tensor_scalar(m[:], sg[:, 0:1], 1.0 / cnt, None, ALU.mult)
    nc.vector.tensor_tensor(t0[:], m[:], m[:], ALU.mult)          # m^2
    nc.vector.tensor_scalar(vpe[:], sg[:, 1:2], 1.0 / cnt, float(EPS), ALU.mult,
                            ALU.add)                              # E[x^2]+eps
    nc.vector.tensor_tensor(vpe[:], vpe[:], t0[:], ALU.subtract)  # var+eps
    # rsqrt with one Newton step (ACT Sqrt is low-precision)
    s0 = pp.tile([128, 1], F32, name=f"bn{tag}_s0")
    y0 = pp.tile([128, 1], F32, name=f"bn{tag}_y0")
    nc.scalar.activation(s0[:], vpe[:], mybir.ActivationFunctionType.Sqrt)
    nc.vector.reciprocal(y0[:], s0[:])
    t1 = pp.tile([128, 1], F32, name=f"bn{tag}_t1")
    nc.vector.tensor_tensor(t1[:], y0[:], y0[:], ALU.mult)
    nc.vector.tensor_tensor(t1[:], t1[:], vpe[:], ALU.mult)
    nc.vector.tensor_scalar(t1[:], t1[:], -0.5, 1.5, ALU.mult, ALU.add)
    nc.vector.tensor_tensor(y0[:], y0[:], t1[:], ALU.mult)        # refined rsqrt
    sc = pp.tile([128, 1], F32, name=f"bn{tag}_sc")
    bi = pp.tile([128, 1], F32, name=f"bn{tag}_bi")
    nc.vector.tensor_tensor(sc[:], y0[:], g_sb[:], ALU.mult)
    nc.vector.tensor_tensor(t0[:], m[:], sc[:], ALU.mult)
    nc.vector.tensor_tensor(bi[:], b_sb[:], t0[:], ALU.subtract)
    return sc, bi


# ---------------------------------------------------------------- entry point
_CACHE = {}


def kernel(x, conv1_w, conv1_b, bn1_g, bn1_b, conv2_w, conv2_b, bn2_g, bn2_b,
           deconv_w, deconv_b):
    world = N_CORES
    if "nc" not in _CACHE:
        _CACHE["nc"] = build_nc(world)
    nc = _CACHE["nc"]

    consts = pack_consts(np.asarray(conv1_w), np.asarray(conv2_w),
                         np.asarray(deconv_w), np.asarray(deconv_b),
                         np.asarray(bn1_g), np.asarray(bn1_b),
                         np.asarray(bn2_g), np.asarray(bn2_b))
    x = np.asarray(x)
    # host-side prep: f16 + even/odd w split -> [c, parity, h, 128];
    # the odd plane carries the D0/D1 factor of the DWT-W chain
    xs = np.stack([x[:, :, :, 0::2].astype(np.float16),
                   (x[:, :, :, 1::2] * (DEC[0] / DEC[1])).astype(np.float16)],
                  axis=2)
    in_maps = []
    for n in range(world):
        m = {"x": np.ascontiguousarray(xs[n])}
        m.update(consts)
        in_maps.append(m)

    res = run_bass_kernel_spmd(
        nc, in_maps, core_ids=list(range(world)),
        trace=bool(int(os.environ.get("WK_TRACE", "0"))))
    out = np.stack(
        [np.concatenate([r[f"out{i}"] for i in range(16)], axis=0).transpose(1, 0, 2)
         for r in res.results], axis=0).astype(np.float32)
    _CACHE["last_perf"] = res
    return out
